# revision 41
# baseline (speedup 1.0000x reference)
"""Multi-head attention block on 8 Trainium2 NeuronCores.

Problem: B=8, N=1024, E=768, H=12, D=64 attention (QKV proj -> softmax(QK^T/8)V
-> output proj), fp32 I/O. Data parallel over batch: core b owns batch b.

v4 design (split-fp8 DoubleRow QKV + all-bf16 attention, host preprocessing):
  - Host precomputes transposed split-fp8 x (xh+xl ~= 32*x^T) and split-fp8
    W_qkv (wh+wl ~= 1024*W_qkv), bf16 W_proj / bias rows. All device loads are
    plain HWDGE DMAs (no casts, no SWDGE descriptor generation, no PE
    transposes of x).
  - QKV projection: 9 fp8 DoubleRow matmuls per psum tile ((xh+xl)@(wh+wl)
    with the xl@wl term dropped), 256-deep contraction pairs at 0.5 cyc/row.
    Psum carries 2^15 scale; Q/K evac rescales (tensor_scalar mult+add bias),
    V keeps the scale which cancels against the 2^15 ones-column in Z.
  - S^T[k,q] per head: two 512-wide bf16 matmuls into a [128,1024] psum
    (contraction d=64 at partition base (h%2)*64); exp on Act -> bf16 expS.
  - U restructured: stationary = expS chunk [128k,128q], moving = V [128k,65]
    (64 dims + 2^15 ones column) -> psum U^T[q,65] accumulated over k chunks;
    invZ = reciprocal of column 64 is a per-partition scalar; attn = U*invZ
    is one DVE tensor_scalar op. Halves U's PE rows vs the classic layout
    and kills the PE invZ broadcast.
  - attn rows (token-major) -> attnT (feature-major) via HWDGE XBAR DMA
    transposes (3 [128,128] bf16 blocks per DMA, zero PE cost).
  - Output proj split: attnT blocks 0..2 projected during late attention as
    PE filler; blocks 3..5 in the tail, accumulated into the same SBUF tile.
  - Emission interleaves S psum fills with QK/V/U/proj filler units so the
    Act engine (exp is ~100us of work, the secondary wall) starves as little
    as possible while PE (the primary wall) stays busy.
"""
import numpy as np

B, N, E, H, D = 8, 1024, 768, 12, 64
SCALE = D ** -0.5
NT = N // 128   # token chunks (8)
NE = E // 128   # embed chunks (6)
NQ = N // 512   # moving-dim tiles (2)
NFS = [(0, 512), (512, 256)]  # free-dim split of E for matmuls
PROJ_SPLIT = 3  # attnT blocks 0..2 in projA (during attention), 3..5 in tail


def _build():
    import concourse.bacc as bacc
    import concourse.mybir as mybir
    import concourse.tile as tile

    F32 = mybir.dt.float32
    BF16 = mybir.dt.bfloat16
    F8 = mybir.dt.float8e4
    EXP = mybir.ActivationFunctionType.Exp
    DR = mybir.MatmulPerfMode.DoubleRow
    MUL = mybir.AluOpType.mult
    ADD = mybir.AluOpType.add

    nc = bacc.Bacc("TRN2", target_bir_lowering=False)
    xh_d = nc.declare_dram_parameter("xh", [E, N], F8, isOutput=False)
    xl_d = nc.declare_dram_parameter("xl", [E, N], F8, isOutput=False)
    wqkvh_d = nc.declare_dram_parameter("W_qkvh", [E, 3 * E], F8, isOutput=False)
    wqkvl_d = nc.declare_dram_parameter("W_qkvl", [E, 3 * E], F8, isOutput=False)
    bqkv_d = nc.declare_dram_parameter("b_qkv", [3 * E], F32, isOutput=False)
    bv_d = nc.declare_dram_parameter("b_v", [E], BF16, isOutput=False)
    wprojh_d = nc.declare_dram_parameter("W_projh", [E, E], F8, isOutput=False)
    wprojl_d = nc.declare_dram_parameter("W_projl", [E, E], F8, isOutput=False)
    wpb45_d = nc.declare_dram_parameter("W_pb45", [256, E], BF16, isOutput=False)
    bp_d = nc.declare_dram_parameter("b_pb", [E], BF16, isOutput=False)
    out_d = nc.declare_dram_parameter("out", [N, E], F32, isOutput=True)

    with tile.TileContext(nc) as tc:
        with (
            tc.tile_pool(name="const", bufs=1) as cp,
            tc.tile_pool(name="main", bufs=1) as qp,
            tc.tile_pool(name="psum", bufs=1, space="PSUM") as ps,
        ):
            # ---- constants ----
            ones1 = cp.tile([1, 128], BF16)
            nc.vector.memset(ones1, 1.0)
            ones32k = cp.tile([1, 128], BF16)
            nc.vector.memset(ones32k, 32768.0)
            bqc = cp.tile([128, 12], F32)   # column fc = b_qkv[128fc:128(fc+1)]

            # ---- long-lived tensors ----
            qT = [qp.tile([128, N], BF16, name=f"qT{c}", tag=f"qT{c}")
                  for c in range(6)]
            kT = [qp.tile([128, N], BF16, name=f"kT{c}", tag=f"kT{c}")
                  for c in range(6)]
            vS = [qp.tile([128, 65 * H], BF16, name=f"vS{i}", tag=f"vS{i}")
                  for i in range(NT)]
            attnS = [qp.tile([128, E], BF16, name=f"atS{i}", tag=f"atS{i}")
                     for i in range(NT)]
            attnT = qp.tile([128, NE * N], BF16)  # [128, (c, 1024)] = 32*attn^T
            attnTv = attnT.rearrange("p (c n) -> p c n", n=N)
            attnTh = qp.tile([128, NE * N], F8)
            attnThv = attnTh.rearrange("p (c n) -> p c n", n=N)
            attnTl = qp.tile([128, NE * N], F8)
            attnTlv = attnTl.rearrange("p (c n) -> p c n", n=N)
            # W_proj pair tiles: pair pp = rows 256pp..256pp+256 as [128, 2, E]
            wphB = qp.tile([128, 2 * 2 * E], F8)
            wplB = qp.tile([128, 2 * 2 * E], F8)
            wphv = wphB.rearrange("p (pp t f) -> p pp t f", t=2, f=E)
            wplv = wplB.rearrange("p (pp t f) -> p pp t f", t=2, f=E)
            wpb45 = qp.tile([128, 2 * E], BF16)
            wpb45v = wpb45.rearrange("p (c f) -> p c f", f=E)
            bv_bc = qp.tile([128, E], F32)
            bp_bc = qp.tile([128, E], F32)
            bv_row = qp.tile([1, E], BF16)
            bp_row = qp.tile([1, E], BF16)
            o_acc = [qp.tile([128, E], F32, name=f"oa{i}", tag=f"oa{i}")
                     for i in range(NT)]

            # expS pool: [128, N] bf16 tiles; 3 heads alive (lag 2)
            ep = tc.alloc_tile_pool(name="exp", bufs=1)
            iz = tc.alloc_tile_pool(name="iz", bufs=1)

            # scoped pool: x / W_qkv fp8 tiles, released once QKV is done
            xp = tc.alloc_tile_pool(name="xw", bufs=1)
            xH = xp.tile([128, NE * N], F8)   # [128, (j, 1024 tok)] = 32*x^T
            xL = xp.tile([128, NE * N], F8)
            xHv = xH.rearrange("p (j n) -> p j n", n=N)
            xLv = xL.rearrange("p (j n) -> p j n", n=N)
            # weight pair big tiles: [128, (p, t, f)] with pair p = W rows
            # 256p..256p+256 split as 2 k-subtiles t
            wqkhB = xp.tile([128, 3 * 2 * 1536], F8)
            wqklB = xp.tile([128, 3 * 2 * 1536], F8)
            wqkhv = wqkhB.rearrange("p (pp t f) -> p pp t f", t=2, f=1536)
            wqklv = wqklB.rearrange("p (pp t f) -> p pp t f", t=2, f=1536)
            wvhB = xp.tile([128, 3 * 2 * E], F8)
            wvlB = xp.tile([128, 3 * 2 * E], F8)
            wvhv = wvhB.rearrange("p (pp t f) -> p pp t f", t=2, f=E)
            wvlv = wvlB.rearrange("p (pp t f) -> p pp t f", t=2, f=E)

            # ---- DMAs (all HWDGE, no casts): few big transfers, with two
            # small priority slices so the first S unit starts early ----
            def wqk_slice(wview, w_d, c0, cw):
                nc.sync.dma_start(
                    out=wview[:, :, :, c0:c0 + cw],
                    in_=w_d[0:768, c0:c0 + cw].rearrange(
                        "(pp t k) f -> k pp t f", t=2, k=128))

            # 1. t=0 / t=6 weight columns (gate the first S unit)
            wqk_slice(wqkhv, wqkvh_d, 0, 128)
            wqk_slice(wqkhv, wqkvh_d, 768, 128)
            wqk_slice(wqklv, wqkvl_d, 0, 128)
            wqk_slice(wqklv, wqkvl_d, 768, 128)
            nc.sync.dma_start(
                out=bqc, in_=bqkv_d[0:1536].rearrange("(f p) -> p f", p=128))
            # 2. x token-half 0, then half 1
            for half in range(2):
                t0 = half * 512
                nc.sync.dma_start(
                    out=xHv[:, :, t0:t0 + 512],
                    in_=xh_d[:, t0:t0 + 512].rearrange(
                        "(j k) n -> k j n", k=128))
                nc.sync.dma_start(
                    out=xLv[:, :, t0:t0 + 512],
                    in_=xl_d[:, t0:t0 + 512].rearrange(
                        "(j k) n -> k j n", k=128))
            nc.sync.dma_start(
                out=bv_row, in_=bv_d[:].rearrange("(o f) -> o f", o=1))
            nc.sync.dma_start(
                out=bp_row, in_=bp_d[:].rearrange("(o f) -> o f", o=1))
            # 3. remaining wqk columns
            wqk_slice(wqkhv, wqkvh_d, 128, 640)
            wqk_slice(wqklv, wqkvl_d, 128, 640)
            wqk_slice(wqkhv, wqkvh_d, 896, 640)
            wqk_slice(wqklv, wqkvl_d, 896, 640)
            # 4. V weights, bias rows, proj weights
            for w_t, w_d in ((wvhB, wqkvh_d), (wvlB, wqkvl_d)):
                nc.sync.dma_start(
                    out=w_t.rearrange("p (pp t f) -> p pp t f", t=2, f=E),
                    in_=w_d[0:768, 1536:].rearrange(
                        "(pp t k) f -> k pp t f", t=2, k=128))
            for w_t, w_d in ((wphB, wprojh_d), (wplB, wprojl_d)):
                nc.sync.dma_start(
                    out=w_t.rearrange("p (pp t f) -> p pp t f", t=2, f=E),
                    in_=w_d[0:512, :].rearrange(
                        "(pp t k) f -> k pp t f", t=2, k=128))
            nc.sync.dma_start(
                out=wpb45.rearrange("p (c f) -> p c f", f=E),
                in_=wpb45_d[:].rearrange("(c k) f -> k c f", k=128))

            def emit_prelude():
                for nf, (f0, fw) in enumerate(NFS):
                    pbv = ps.tile([128, 512], F32, name=f"pbv{nf}", tag="mm",
                                  bufs=2)
                    nc.tensor.matmul(pbv[:, :fw], ones32k,
                                     bv_row[:, f0:f0 + fw],
                                     start=True, stop=True)
                    nc.vector.tensor_copy(bv_bc[:, f0:f0 + fw], pbv[:, :fw])
                    pbp = ps.tile([128, 512], F32, name=f"pbp{nf}", tag="mm",
                                  bufs=2)
                    nc.tensor.matmul(pbp[:, :fw], ones32k,
                                     bp_row[:, f0:f0 + fw],
                                     start=True, stop=True)
                    nc.vector.tensor_copy(bp_bc[:, f0:f0 + fw], pbp[:, :fw])
                for i in range(NT):
                    nc.vector.memset(
                        vS[i].rearrange("p (h c) -> p h c", c=65)[:, :, 64:65],
                        1024.0)

            # ================= emission units =================
            def emit_qk_unit(t, q, evac_act=False):
                """One (feature-tile, 512-token-half) of Q or K projection.
                Split-fp8 DoubleRow: (xh+xl)@(wh+wl), xl@wl dropped."""
                dst = qT[t] if t < 6 else kT[t - 6]
                wcol0 = t * 128
                pq = ps.tile([128, 512], F32, name=f"pq{t}_{q}", tag="mm",
                             bufs=2)
                terms = [(wqkhv, xHv), (wqkhv, xLv), (wqklv, xHv)]
                nmm = 3 * len(terms)
                mi = 0
                for p in range(3):
                    for w_v, x_v in terms:
                        nc.tensor.matmul(
                            pq,
                            w_v[:, p, :, wcol0:wcol0 + 128],
                            x_v[:, 2 * p:2 * p + 2, q * 512:(q + 1) * 512],
                            start=(mi == 0), stop=(mi == nmm - 1),
                            perf_mode=DR)
                        mi += 1
                if evac_act:
                    nc.scalar.activation(
                        dst[:, q * 512:(q + 1) * 512], pq,
                        mybir.ActivationFunctionType.Identity,
                        bias=bqc[:, t:t + 1], scale=float(2.0 ** -15))
                else:
                    nc.vector.tensor_scalar(
                        out=dst[:, q * 512:(q + 1) * 512], in0=pq,
                        scalar1=float(2.0 ** -15), scalar2=bqc[:, t:t + 1],
                        op0=MUL, op1=ADD)

            def emit_v_unit(i, nf):
                """One (token-chunk, free-half) of the V projection."""
                f0, fw = NFS[nf]
                pv = ps.tile([128, 512], F32, name=f"pv{i}_{nf}", tag="mm",
                             bufs=2)
                terms = [(xHv, wvhv), (xHv, wvlv), (xLv, wvhv)]
                nmm = 3 * len(terms)
                mi = 0
                for p in range(3):
                    for x_v, w_v in terms:
                        nc.tensor.matmul(
                            pv[:, :fw],
                            x_v[:, 2 * p:2 * p + 2, i * 128:(i + 1) * 128],
                            w_v[:, p, :, f0:f0 + fw],
                            start=(mi == 0), stop=(mi == nmm - 1),
                            perf_mode=DR)
                        mi += 1
                nh, h0 = fw // D, f0 // D
                nc.vector.tensor_add(
                    vS[i].rearrange("p (h c) -> p h c", c=65)
                        [:, h0:h0 + nh, 0:64],
                    pv[:, :fw].rearrange("p (h d) -> p h d", d=D),
                    bv_bc[:, f0:f0 + fw].rearrange("p (h d) -> p h d", d=D))

            expS_of = {}

            def emit_s_unit(h, kc):
                """S^T[k-chunk, all q] for one head + exp -> bf16 expS."""
                c, r0 = h // 2, (h % 2) * 64
                if kc == 0:
                    expS_of[h] = [
                        ep.tile([128, N], BF16, name=f"eS{h}_{k2}",
                                tag="expS", bufs=24)
                        for k2 in range(NT)]
                pss = ps.tile([128, N], F32, name=f"ps{h}_{kc}", tag="s2",
                              bufs=2)
                for q in range(NQ):
                    nc.tensor.matmul(
                        pss[:, q * 512:(q + 1) * 512],
                        kT[c][r0:r0 + 64, kc * 128:(kc + 1) * 128],
                        qT[c][r0:r0 + 64, q * 512:(q + 1) * 512],
                        start=True, stop=True)
                nc.scalar.activation(expS_of[h][kc], pss, EXP,
                                     scale=float(SCALE))

            def emit_u_unit(h, qc, evac_act=False):
                """U^T[q-chunk, 65] for one head; normalize into attnS."""
                expS = expS_of[h]
                pu = ps.tile([128, 512], F32, name=f"pu{h}_{qc}", tag="u",
                             bufs=2)
                for kc in range(NT):
                    nc.tensor.matmul(
                        pu[:, 0:65],
                        expS[kc][:, qc * 128:(qc + 1) * 128],
                        vS[kc][:, h * 65:h * 65 + 65],
                        start=(kc == 0), stop=(kc == NT - 1))
                invz = iz.tile([128, 1], F32, name=f"iv{h}_{qc}", tag="iz",
                               bufs=3)
                nc.vector.reciprocal(invz, pu[:, 64:65])
                if evac_act:
                    nc.scalar.activation(
                        attnS[qc][:, h * D:(h + 1) * D], pu[:, 0:64],
                        mybir.ActivationFunctionType.Copy, scale=invz)
                else:
                    nc.vector.tensor_scalar_mul(
                        attnS[qc][:, h * D:(h + 1) * D], pu[:, 0:64], invz)

            def emit_transp(qc, pp):
                """DMA-transpose one block pair into attnT (32*attn^T bf16),
                then split to fp8 hi/lo on the Pool engine."""
                c0 = 2 * pp
                t_bf = attnTv[:, c0:c0 + 2, qc * 128:(qc + 1) * 128]
                nc.sync.dma_start_transpose(
                    t_bf, attnS[qc][:, c0 * 128:(c0 + 2) * 128])
                if pp < 2:
                    t_hi = attnThv[:, c0:c0 + 2, qc * 128:(qc + 1) * 128]
                    nc.gpsimd.tensor_copy(t_hi, t_bf)
                    nc.gpsimd.tensor_sub(
                        attnTlv[:, c0:c0 + 2, qc * 128:(qc + 1) * 128],
                        t_bf, t_hi)

            def emit_proj_unit(i, nf, pp):
                """Half-row of output proj for block pair pp (split-fp8 DR).
                pp=0 adds the (2^15-scaled) bias; pp=1 accumulates; pp=2
                accumulates and rescales to the final f32 output."""
                f0, fw = NFS[nf]
                po = ps.tile([128, 512], F32, name=f"po{i}_{nf}_{pp}",
                             tag="mm", bufs=2)
                if pp < 2:
                    terms = [(attnThv, wphv), (attnThv, wplv), (attnTlv, wphv)]
                    for mi, (a_v, w_v) in enumerate(terms):
                        nc.tensor.matmul(
                            po[:, :fw],
                            a_v[:, 2 * pp:2 * pp + 2, i * 128:(i + 1) * 128],
                            w_v[:, pp, :, f0:f0 + fw],
                            start=(mi == 0), stop=(mi == len(terms) - 1),
                            perf_mode=DR)
                else:
                    for ci, cb in enumerate((4, 5)):
                        nc.tensor.matmul(
                            po[:, :fw],
                            attnTv[:, cb, i * 128:(i + 1) * 128],
                            wpb45v[:, cb - 4, f0:f0 + fw],
                            start=(ci == 0), stop=(ci == 1))
                if pp == 0:
                    nc.vector.tensor_add(
                        o_acc[i][:, f0:f0 + fw], po[:, :fw],
                        bp_bc[:, f0:f0 + fw])
                elif pp == 1:
                    nc.vector.tensor_add(
                        o_acc[i][:, f0:f0 + fw], po[:, :fw],
                        o_acc[i][:, f0:f0 + fw])
                    # rescale the pairs-0/1(+bias) partial here, off the tail
                    nc.gpsimd.tensor_scalar_mul(
                        o_acc[i][:, f0:f0 + fw], o_acc[i][:, f0:f0 + fw],
                        float(2.0 ** -15))
                else:
                    nc.vector.tensor_add(
                        o_acc[i][:, f0:f0 + fw], po[:, :fw],
                        o_acc[i][:, f0:f0 + fw])

            # ================= schedule =================
            for q in range(NQ):
                for t in (0, 6):
                    emit_qk_unit(t, q)

            for h in range(H):
                c = h // 2
                fillers = []
                if h == 0:
                    fillers.append(("prelude",))
                if h < 10:  # QK chunk c+1: 2 units during each of h=2c, 2c+1
                    t0 = (c + 1, 6 + c + 1)
                    if h % 2 == 0:
                        fillers.append(("qk", t0[0], 0))
                        fillers.append(("qk", t0[1], 0))
                    else:
                        fillers.append(("qk", t0[0], 1))
                        fillers.append(("qk", t0[1], 1))
                if h in (1, 2):  # V projection (wv lands early now)
                    lo, hi = (0, 3) if h == 1 else (3, 8)
                    for i in range(lo, hi):
                        fillers.append(("v", i, 0))
                        fillers.append(("v", i, 1))
                if h >= 2:  # U for head h-2 (after V units at h=2)
                    for qc in range(NT):
                        fillers.append(("u", h - 2, qc))
                if h == 10:
                    fillers.append(("xfree",))
                if h in (6, 7):  # proj pair 0 (blocks 0,1; ready after U(3))
                    for i in range(4 * (h - 6), 4 * (h - 6) + 4):
                        fillers.append(("pa", i, 0, 0))
                        fillers.append(("pa", i, 1, 0))
                if h in (10, 11):  # proj pair 1 (blocks 2,3; after U(7))
                    for i in range(4 * (h - 10), 4 * (h - 10) + 4):
                        fillers.append(("pa", i, 0, 1))
                        fillers.append(("pa", i, 1, 1))
                if h == 11:  # U(10) late in head 11 (exp(10) done by then)
                    for qc in range(NT):
                        fillers.append(("u", 10, qc))

                fillers.sort(key=lambda f: f[0] == "u")

                def drain(k):
                    for _ in range(k):
                        if not fillers:
                            return
                        f = fillers.pop(0)
                        if f[0] == "v":
                            emit_v_unit(f[1], f[2])
                        elif f[0] == "qk":
                            emit_qk_unit(f[1], f[2])
                        elif f[0] == "u":
                            emit_u_unit(f[1], f[2])
                            if f[1] == 3:
                                emit_transp(f[2], 0)
                            elif f[1] == 7:
                                emit_transp(f[2], 1)
                        elif f[0] == "pa":
                            emit_proj_unit(f[1], f[2], f[3])
                        elif f[0] == "xfree":
                            xp.release()
                        elif f[0] == "prelude":
                            emit_prelude()

                for kc in range(NT):
                    emit_s_unit(h, kc)
                    drain((len(fillers) + NT - kc - 1) // (NT - kc))
                drain(len(fillers))

            # ---- tail: U(11) + attnT blocks 4,5 + proj pair 2, pipelined
            def emit_out(i):
                emit_proj_unit(i, 0, 2)
                emit_proj_unit(i, 1, 2)
                nc.sync.dma_start(
                    out=out_d[i * 128:(i + 1) * 128, :], in_=o_acc[i])

            for qc in range(NT):
                emit_u_unit(11, qc)
                emit_transp(qc, 2)
                if qc >= 1:
                    emit_out(qc - 1)
            emit_out(NT - 1)
            iz.release()
            ep.release()
    nc.compile()
    return nc


_NC_CACHE = None


def kernel(x, W_qkv, b_qkv, W_proj, b_proj):
    from concourse.bass_utils import run_bass_kernel_spmd
    import ml_dtypes

    F8NP = ml_dtypes.float8_e4m3
    BF16NP = ml_dtypes.bfloat16

    global _NC_CACHE
    if _NC_CACHE is None:
        _NC_CACHE = _build()
    nc = _NC_CACHE

    x = np.asarray(x, dtype=np.float32)
    W_qkv = np.asarray(W_qkv, dtype=np.float32)
    b_qkv = np.ascontiguousarray(np.asarray(b_qkv, dtype=np.float32))
    W_proj = np.asarray(W_proj, dtype=np.float32)
    b_proj = np.asarray(b_proj, dtype=np.float32)

    # host-side preprocessing: transposed split-fp8 x, split-fp8 W_qkv,
    # bf16 W_proj and bias rows
    def split8(a):
        hi = a.astype(F8NP)
        lo = (a - hi.astype(np.float32)).astype(F8NP)
        return np.ascontiguousarray(hi), np.ascontiguousarray(lo)

    w_h, w_l = split8(W_qkv * np.float32(1024.0))
    xt32 = np.ascontiguousarray(np.swapaxes(x, 1, 2)) * np.float32(32.0)
    xsplit = [split8(xt32[b]) for b in range(B)]
    wp_h, wp_l = split8(W_proj * np.float32(1024.0))
    wp45 = np.ascontiguousarray(
        (W_proj[512:, :] * np.float32(1.0 / 32.0)).astype(BF16NP))
    bv_b = np.ascontiguousarray(b_qkv[2 * E:].astype(BF16NP))
    bp_b = np.ascontiguousarray(b_proj.astype(BF16NP))

    in_maps = [
        {"xh": xsplit[b][0], "xl": xsplit[b][1],
         "W_qkvh": w_h, "W_qkvl": w_l, "b_qkv": b_qkv,
         "b_v": bv_b, "W_projh": wp_h, "W_projl": wp_l, "W_pb45": wp45,
         "b_pb": bp_b}
        for b in range(B)
    ]
    res = run_bass_kernel_spmd(nc, in_maps, core_ids=list(range(B)))
    return np.stack([np.asarray(res.results[b]["out"]) for b in range(B)])


# revision 42
# speedup vs baseline: 1.0012x; 1.0012x over previous
"""Multi-head attention block on 8 Trainium2 NeuronCores.

Problem: B=8, N=1024, E=768, H=12, D=64 attention (QKV proj -> softmax(QK^T/8)V
-> output proj), fp32 I/O. Data parallel over batch: core b owns batch b.

v4 design (split-fp8 DoubleRow QKV + all-bf16 attention, host preprocessing):
  - Host precomputes transposed split-fp8 x (xh+xl ~= 32*x^T) and split-fp8
    W_qkv (wh+wl ~= 1024*W_qkv), bf16 W_proj / bias rows. All device loads are
    plain HWDGE DMAs (no casts, no SWDGE descriptor generation, no PE
    transposes of x).
  - QKV projection: 9 fp8 DoubleRow matmuls per psum tile ((xh+xl)@(wh+wl)
    with the xl@wl term dropped), 256-deep contraction pairs at 0.5 cyc/row.
    Psum carries 2^15 scale; Q/K evac rescales (tensor_scalar mult+add bias),
    V keeps the scale which cancels against the 2^15 ones-column in Z.
  - S^T[k,q] per head: two 512-wide bf16 matmuls into a [128,1024] psum
    (contraction d=64 at partition base (h%2)*64); exp on Act -> bf16 expS.
  - U restructured: stationary = expS chunk [128k,128q], moving = V [128k,65]
    (64 dims + 2^15 ones column) -> psum U^T[q,65] accumulated over k chunks;
    invZ = reciprocal of column 64 is a per-partition scalar; attn = U*invZ
    is one DVE tensor_scalar op. Halves U's PE rows vs the classic layout
    and kills the PE invZ broadcast.
  - attn rows (token-major) -> attnT (feature-major) via HWDGE XBAR DMA
    transposes (3 [128,128] bf16 blocks per DMA, zero PE cost).
  - Output proj split: attnT blocks 0..2 projected during late attention as
    PE filler; blocks 3..5 in the tail, accumulated into the same SBUF tile.
  - Emission interleaves S psum fills with QK/V/U/proj filler units so the
    Act engine (exp is ~100us of work, the secondary wall) starves as little
    as possible while PE (the primary wall) stays busy.
"""
import numpy as np

B, N, E, H, D = 8, 1024, 768, 12, 64
SCALE = D ** -0.5
NT = N // 128   # token chunks (8)
NE = E // 128   # embed chunks (6)
NQ = N // 512   # moving-dim tiles (2)
NFS = [(0, 512), (512, 256)]  # free-dim split of E for matmuls
PROJ_SPLIT = 3  # attnT blocks 0..2 in projA (during attention), 3..5 in tail


def _build():
    import concourse.bacc as bacc
    import concourse.mybir as mybir
    import concourse.tile as tile

    F32 = mybir.dt.float32
    BF16 = mybir.dt.bfloat16
    F8 = mybir.dt.float8e4
    EXP = mybir.ActivationFunctionType.Exp
    DR = mybir.MatmulPerfMode.DoubleRow
    MUL = mybir.AluOpType.mult
    ADD = mybir.AluOpType.add

    nc = bacc.Bacc("TRN2", target_bir_lowering=False)
    xh_d = nc.declare_dram_parameter("xh", [E, N], F8, isOutput=False)
    xl_d = nc.declare_dram_parameter("xl", [E, N], F8, isOutput=False)
    wqkvh_d = nc.declare_dram_parameter("W_qkvh", [E, 3 * E], F8, isOutput=False)
    wqkvl_d = nc.declare_dram_parameter("W_qkvl", [E, 3 * E], F8, isOutput=False)
    bqkv_d = nc.declare_dram_parameter("b_qkv", [3 * E], F32, isOutput=False)
    bv_d = nc.declare_dram_parameter("b_v", [E], BF16, isOutput=False)
    wprojh_d = nc.declare_dram_parameter("W_projh", [E, E], F8, isOutput=False)
    wprojl_d = nc.declare_dram_parameter("W_projl", [E, E], F8, isOutput=False)
    wpb45_d = nc.declare_dram_parameter("W_pb45", [256, E], BF16, isOutput=False)
    bp_d = nc.declare_dram_parameter("b_pb", [E], BF16, isOutput=False)
    out_d = nc.declare_dram_parameter("out", [N, E], F32, isOutput=True)

    with tile.TileContext(nc) as tc:
        with (
            tc.tile_pool(name="const", bufs=1) as cp,
            tc.tile_pool(name="main", bufs=1) as qp,
            tc.tile_pool(name="psum", bufs=1, space="PSUM") as ps,
        ):
            # ---- constants ----
            ones1 = cp.tile([1, 128], BF16)
            nc.vector.memset(ones1, 1.0)
            ones32k = cp.tile([1, 128], BF16)
            nc.vector.memset(ones32k, 32768.0)
            bqc = cp.tile([128, 12], F32)   # column fc = b_qkv[128fc:128(fc+1)]

            # ---- long-lived tensors ----
            qT = [qp.tile([128, N], BF16, name=f"qT{c}", tag=f"qT{c}")
                  for c in range(6)]
            kT = [qp.tile([128, N], BF16, name=f"kT{c}", tag=f"kT{c}")
                  for c in range(6)]
            vS = [qp.tile([128, 65 * H], BF16, name=f"vS{i}", tag=f"vS{i}")
                  for i in range(NT)]
            attnS = [qp.tile([128, E], BF16, name=f"atS{i}", tag=f"atS{i}")
                     for i in range(NT)]
            attnT = qp.tile([128, NE * N], BF16)  # [128, (c, 1024)] = 32*attn^T
            attnTv = attnT.rearrange("p (c n) -> p c n", n=N)
            attnTh = qp.tile([128, NE * N], F8)
            attnThv = attnTh.rearrange("p (c n) -> p c n", n=N)
            attnTl = qp.tile([128, NE * N], F8)
            attnTlv = attnTl.rearrange("p (c n) -> p c n", n=N)
            # W_proj pair tiles: pair pp = rows 256pp..256pp+256 as [128, 2, E]
            wphB = qp.tile([128, 2 * 2 * E], F8)
            wplB = qp.tile([128, 2 * 2 * E], F8)
            wphv = wphB.rearrange("p (pp t f) -> p pp t f", t=2, f=E)
            wplv = wplB.rearrange("p (pp t f) -> p pp t f", t=2, f=E)
            wpb45 = qp.tile([128, 2 * E], BF16)
            wpb45v = wpb45.rearrange("p (c f) -> p c f", f=E)
            bv_bc = qp.tile([128, E], F32)
            bp_bc = qp.tile([128, E], F32)
            bv_row = qp.tile([1, E], BF16)
            bp_row = qp.tile([1, E], BF16)
            o_acc = [qp.tile([128, E], F32, name=f"oa{i}", tag=f"oa{i}")
                     for i in range(NT)]

            # expS pool: [128, N] bf16 tiles; 3 heads alive (lag 2)
            ep = tc.alloc_tile_pool(name="exp", bufs=1)
            iz = tc.alloc_tile_pool(name="iz", bufs=1)

            # scoped pool: x / W_qkv fp8 tiles, released once QKV is done
            xp = tc.alloc_tile_pool(name="xw", bufs=1)
            xH = xp.tile([128, NE * N], F8)   # [128, (j, 1024 tok)] = 32*x^T
            xL = xp.tile([128, NE * N], F8)
            xHv = xH.rearrange("p (j n) -> p j n", n=N)
            xLv = xL.rearrange("p (j n) -> p j n", n=N)
            # weight pair big tiles: [128, (p, t, f)] with pair p = W rows
            # 256p..256p+256 split as 2 k-subtiles t
            wqkhB = xp.tile([128, 3 * 2 * 1536], F8)
            wqklB = xp.tile([128, 3 * 2 * 1536], F8)
            wqkhv = wqkhB.rearrange("p (pp t f) -> p pp t f", t=2, f=1536)
            wqklv = wqklB.rearrange("p (pp t f) -> p pp t f", t=2, f=1536)
            wvhB = xp.tile([128, 3 * 2 * E], F8)
            wvlB = xp.tile([128, 3 * 2 * E], F8)
            wvhv = wvhB.rearrange("p (pp t f) -> p pp t f", t=2, f=E)
            wvlv = wvlB.rearrange("p (pp t f) -> p pp t f", t=2, f=E)

            # ---- DMAs (all HWDGE, no casts): few big transfers, with two
            # small priority slices so the first S unit starts early ----
            def wqk_slice(wview, w_d, c0, cw):
                nc.sync.dma_start(
                    out=wview[:, :, :, c0:c0 + cw],
                    in_=w_d[0:768, c0:c0 + cw].rearrange(
                        "(pp t k) f -> k pp t f", t=2, k=128))

            # 1. t=0 / t=6 weight columns (gate the first S unit)
            wqk_slice(wqkhv, wqkvh_d, 0, 128)
            wqk_slice(wqkhv, wqkvh_d, 768, 128)
            wqk_slice(wqklv, wqkvl_d, 0, 128)
            wqk_slice(wqklv, wqkvl_d, 768, 128)
            nc.sync.dma_start(
                out=bqc, in_=bqkv_d[0:1536].rearrange("(f p) -> p f", p=128))
            # 2. x token-half 0, then half 1
            for half in range(2):
                t0 = half * 512
                nc.sync.dma_start(
                    out=xHv[:, :, t0:t0 + 512],
                    in_=xh_d[:, t0:t0 + 512].rearrange(
                        "(j k) n -> k j n", k=128))
                nc.sync.dma_start(
                    out=xLv[:, :, t0:t0 + 512],
                    in_=xl_d[:, t0:t0 + 512].rearrange(
                        "(j k) n -> k j n", k=128))
            nc.sync.dma_start(
                out=bv_row, in_=bv_d[:].rearrange("(o f) -> o f", o=1))
            nc.sync.dma_start(
                out=bp_row, in_=bp_d[:].rearrange("(o f) -> o f", o=1))
            # 3. remaining wqk columns
            wqk_slice(wqkhv, wqkvh_d, 128, 640)
            wqk_slice(wqklv, wqkvl_d, 128, 640)
            wqk_slice(wqkhv, wqkvh_d, 896, 640)
            wqk_slice(wqklv, wqkvl_d, 896, 640)
            # 4. V weights, bias rows, proj weights
            for w_t, w_d in ((wvhB, wqkvh_d), (wvlB, wqkvl_d)):
                nc.sync.dma_start(
                    out=w_t.rearrange("p (pp t f) -> p pp t f", t=2, f=E),
                    in_=w_d[0:768, 1536:].rearrange(
                        "(pp t k) f -> k pp t f", t=2, k=128))
            for w_t, w_d in ((wphB, wprojh_d), (wplB, wprojl_d)):
                nc.sync.dma_start(
                    out=w_t.rearrange("p (pp t f) -> p pp t f", t=2, f=E),
                    in_=w_d[0:512, :].rearrange(
                        "(pp t k) f -> k pp t f", t=2, k=128))
            nc.sync.dma_start(
                out=wpb45.rearrange("p (c f) -> p c f", f=E),
                in_=wpb45_d[:].rearrange("(c k) f -> k c f", k=128))

            def emit_prelude():
                for nf, (f0, fw) in enumerate(NFS):
                    pbv = ps.tile([128, 512], F32, name=f"pbv{nf}", tag="mm",
                                  bufs=2)
                    nc.tensor.matmul(pbv[:, :fw], ones32k,
                                     bv_row[:, f0:f0 + fw],
                                     start=True, stop=True)
                    nc.vector.tensor_copy(bv_bc[:, f0:f0 + fw], pbv[:, :fw])
                    pbp = ps.tile([128, 512], F32, name=f"pbp{nf}", tag="mm",
                                  bufs=2)
                    nc.tensor.matmul(pbp[:, :fw], ones32k,
                                     bp_row[:, f0:f0 + fw],
                                     start=True, stop=True)
                    nc.vector.tensor_copy(bp_bc[:, f0:f0 + fw], pbp[:, :fw])
                for i in range(NT):
                    nc.vector.memset(
                        vS[i].rearrange("p (h c) -> p h c", c=65)[:, :, 64:65],
                        1024.0)

            # ================= emission units =================
            def emit_qk_unit(t, q, evac_act=False):
                """One (feature-tile, 512-token-half) of Q or K projection.
                Split-fp8 DoubleRow: (xh+xl)@(wh+wl), xl@wl dropped."""
                dst = qT[t] if t < 6 else kT[t - 6]
                wcol0 = t * 128
                pq = ps.tile([128, 512], F32, name=f"pq{t}_{q}", tag="mm",
                             bufs=2)
                terms = [(wqkhv, xHv), (wqkhv, xLv), (wqklv, xHv)]
                nmm = 3 * len(terms)
                mi = 0
                for p in range(3):
                    for w_v, x_v in terms:
                        nc.tensor.matmul(
                            pq,
                            w_v[:, p, :, wcol0:wcol0 + 128],
                            x_v[:, 2 * p:2 * p + 2, q * 512:(q + 1) * 512],
                            start=(mi == 0), stop=(mi == nmm - 1),
                            perf_mode=DR)
                        mi += 1
                if evac_act:
                    nc.scalar.activation(
                        dst[:, q * 512:(q + 1) * 512], pq,
                        mybir.ActivationFunctionType.Identity,
                        bias=bqc[:, t:t + 1], scale=float(2.0 ** -15))
                else:
                    nc.vector.tensor_scalar(
                        out=dst[:, q * 512:(q + 1) * 512], in0=pq,
                        scalar1=float(2.0 ** -15), scalar2=bqc[:, t:t + 1],
                        op0=MUL, op1=ADD)

            def emit_v_unit(i, nf):
                """One (token-chunk, free-half) of the V projection."""
                f0, fw = NFS[nf]
                pv = ps.tile([128, 512], F32, name=f"pv{i}_{nf}", tag="mm",
                             bufs=2)
                terms = [(xHv, wvhv), (xHv, wvlv), (xLv, wvhv)]
                nmm = 3 * len(terms)
                mi = 0
                for p in range(3):
                    for x_v, w_v in terms:
                        nc.tensor.matmul(
                            pv[:, :fw],
                            x_v[:, 2 * p:2 * p + 2, i * 128:(i + 1) * 128],
                            w_v[:, p, :, f0:f0 + fw],
                            start=(mi == 0), stop=(mi == nmm - 1),
                            perf_mode=DR)
                        mi += 1
                nh, h0 = fw // D, f0 // D
                nc.vector.tensor_add(
                    vS[i].rearrange("p (h c) -> p h c", c=65)
                        [:, h0:h0 + nh, 0:64],
                    pv[:, :fw].rearrange("p (h d) -> p h d", d=D),
                    bv_bc[:, f0:f0 + fw].rearrange("p (h d) -> p h d", d=D))

            expS_of = {}

            def emit_s_unit(h, kc):
                """S^T[k-chunk, all q] for one head + exp -> bf16 expS."""
                c, r0 = h // 2, (h % 2) * 64
                if kc == 0:
                    expS_of[h] = [
                        ep.tile([128, N], BF16, name=f"eS{h}_{k2}",
                                tag="expS", bufs=24)
                        for k2 in range(NT)]
                pss = ps.tile([128, N], F32, name=f"ps{h}_{kc}", tag="s2",
                              bufs=2)
                for q in range(NQ):
                    nc.tensor.matmul(
                        pss[:, q * 512:(q + 1) * 512],
                        kT[c][r0:r0 + 64, kc * 128:(kc + 1) * 128],
                        qT[c][r0:r0 + 64, q * 512:(q + 1) * 512],
                        start=True, stop=True)
                nc.scalar.activation(expS_of[h][kc], pss, EXP,
                                     scale=float(SCALE))

            def emit_u_unit(h, qc, evac_act=False):
                """U^T[q-chunk, 65] for one head; normalize into attnS."""
                expS = expS_of[h]
                pu = ps.tile([128, 512], F32, name=f"pu{h}_{qc}", tag="u",
                             bufs=2)
                for kc in range(NT):
                    nc.tensor.matmul(
                        pu[:, 0:65],
                        expS[kc][:, qc * 128:(qc + 1) * 128],
                        vS[kc][:, h * 65:h * 65 + 65],
                        start=(kc == 0), stop=(kc == NT - 1))
                invz = iz.tile([128, 1], F32, name=f"iv{h}_{qc}", tag="iz",
                               bufs=3)
                nc.vector.reciprocal(invz, pu[:, 64:65])
                if evac_act:
                    nc.scalar.activation(
                        attnS[qc][:, h * D:(h + 1) * D], pu[:, 0:64],
                        mybir.ActivationFunctionType.Copy, scale=invz)
                else:
                    nc.vector.tensor_scalar_mul(
                        attnS[qc][:, h * D:(h + 1) * D], pu[:, 0:64], invz)

            def emit_transp(qc, pp):
                """DMA-transpose one block pair into attnT (32*attn^T bf16),
                then split to fp8 hi/lo on the Pool engine."""
                c0 = 2 * pp
                t_bf = attnTv[:, c0:c0 + 2, qc * 128:(qc + 1) * 128]
                nc.sync.dma_start_transpose(
                    t_bf, attnS[qc][:, c0 * 128:(c0 + 2) * 128])
                if pp < 2:
                    t_hi = attnThv[:, c0:c0 + 2, qc * 128:(qc + 1) * 128]
                    nc.gpsimd.tensor_copy(t_hi, t_bf)
                    nc.gpsimd.tensor_sub(
                        attnTlv[:, c0:c0 + 2, qc * 128:(qc + 1) * 128],
                        t_bf, t_hi)

            def emit_proj_unit(i, nf, pp):
                """Half-row of output proj for block pair pp (split-fp8 DR).
                pp=0 adds the (2^15-scaled) bias; pp=1 accumulates; pp=2
                accumulates and rescales to the final f32 output."""
                f0, fw = NFS[nf]
                po = ps.tile([128, 512], F32, name=f"po{i}_{nf}_{pp}",
                             tag="mm", bufs=2)
                if pp < 2:
                    terms = [(attnThv, wphv), (attnThv, wplv), (attnTlv, wphv)]
                    for mi, (a_v, w_v) in enumerate(terms):
                        nc.tensor.matmul(
                            po[:, :fw],
                            a_v[:, 2 * pp:2 * pp + 2, i * 128:(i + 1) * 128],
                            w_v[:, pp, :, f0:f0 + fw],
                            start=(mi == 0), stop=(mi == len(terms) - 1),
                            perf_mode=DR)
                else:
                    for ci, cb in enumerate((4, 5)):
                        nc.tensor.matmul(
                            po[:, :fw],
                            attnTv[:, cb, i * 128:(i + 1) * 128],
                            wpb45v[:, cb - 4, f0:f0 + fw],
                            start=(ci == 0), stop=(ci == 1))
                if pp == 0:
                    nc.vector.tensor_add(
                        o_acc[i][:, f0:f0 + fw], po[:, :fw],
                        bp_bc[:, f0:f0 + fw])
                elif pp == 1:
                    nc.vector.tensor_add(
                        o_acc[i][:, f0:f0 + fw], po[:, :fw],
                        o_acc[i][:, f0:f0 + fw])
                    # rescale the pairs-0/1(+bias) partial here, off the tail
                    nc.gpsimd.tensor_scalar_mul(
                        o_acc[i][:, f0:f0 + fw], o_acc[i][:, f0:f0 + fw],
                        float(2.0 ** -15))
                else:
                    nc.vector.tensor_add(
                        o_acc[i][:, f0:f0 + fw], po[:, :fw],
                        o_acc[i][:, f0:f0 + fw])

            # ================= schedule =================
            for q in range(NQ):
                for t in (0, 6):
                    emit_qk_unit(t, q)

            for h in range(H):
                c = h // 2
                fillers = []
                if h == 0:
                    fillers.append(("prelude",))
                if h < 10:  # QK chunk c+1: 2 units during each of h=2c, 2c+1
                    t0 = (c + 1, 6 + c + 1)
                    if h % 2 == 0:
                        fillers.append(("qk", t0[0], 0))
                        fillers.append(("qk", t0[1], 0))
                    else:
                        fillers.append(("qk", t0[0], 1))
                        fillers.append(("qk", t0[1], 1))
                if h in (1, 2):  # V projection (wv lands early now)
                    lo, hi = (0, 3) if h == 1 else (3, 8)
                    for i in range(lo, hi):
                        fillers.append(("v", i, 0))
                        fillers.append(("v", i, 1))
                if h >= 2:  # U for head h-2 (after V units at h=2)
                    for qc in range(NT):
                        fillers.append(("u", h - 2, qc))
                if h == 10:
                    fillers.append(("xfree",))
                if h in (6, 7):  # proj pair 0 (blocks 0,1; ready after U(3))
                    for i in range(4 * (h - 6), 4 * (h - 6) + 4):
                        fillers.append(("pa", i, 0, 0))
                        fillers.append(("pa", i, 1, 0))
                if h in (10, 11):  # proj pair 1 (blocks 2,3; after U(7))
                    for i in range(4 * (h - 10), 4 * (h - 10) + 4):
                        fillers.append(("pa", i, 0, 1))
                        fillers.append(("pa", i, 1, 1))
                if h == 11:  # U(10) late in head 11 (exp(10) done by then)
                    for qc in range(NT):
                        fillers.append(("u", 10, qc))

                fillers.sort(
                    key=lambda f: (f[0] == "u", f[0] == "prelude"))

                def drain(k):
                    for _ in range(k):
                        if not fillers:
                            return
                        f = fillers.pop(0)
                        if f[0] == "v":
                            emit_v_unit(f[1], f[2])
                        elif f[0] == "qk":
                            emit_qk_unit(f[1], f[2])
                        elif f[0] == "u":
                            emit_u_unit(f[1], f[2])
                            if f[1] == 3:
                                emit_transp(f[2], 0)
                            elif f[1] == 7:
                                emit_transp(f[2], 1)
                        elif f[0] == "pa":
                            emit_proj_unit(f[1], f[2], f[3])
                        elif f[0] == "xfree":
                            xp.release()
                        elif f[0] == "prelude":
                            emit_prelude()

                for kc in range(NT):
                    emit_s_unit(h, kc)
                    drain((len(fillers) + NT - kc - 1) // (NT - kc))
                drain(len(fillers))

            # ---- tail: U(11) + attnT blocks 4,5 + proj pair 2, pipelined
            def emit_out(i):
                emit_proj_unit(i, 0, 2)
                emit_proj_unit(i, 1, 2)
                nc.sync.dma_start(
                    out=out_d[i * 128:(i + 1) * 128, :], in_=o_acc[i])

            for qc in range(NT):
                emit_u_unit(11, qc)
                emit_transp(qc, 2)
                if qc >= 1:
                    emit_out(qc - 1)
            emit_out(NT - 1)
            iz.release()
            ep.release()
    nc.compile()
    return nc


_NC_CACHE = None


def kernel(x, W_qkv, b_qkv, W_proj, b_proj):
    from concourse.bass_utils import run_bass_kernel_spmd
    import ml_dtypes

    F8NP = ml_dtypes.float8_e4m3
    BF16NP = ml_dtypes.bfloat16

    global _NC_CACHE
    if _NC_CACHE is None:
        _NC_CACHE = _build()
    nc = _NC_CACHE

    x = np.asarray(x, dtype=np.float32)
    W_qkv = np.asarray(W_qkv, dtype=np.float32)
    b_qkv = np.ascontiguousarray(np.asarray(b_qkv, dtype=np.float32))
    W_proj = np.asarray(W_proj, dtype=np.float32)
    b_proj = np.asarray(b_proj, dtype=np.float32)

    # host-side preprocessing: transposed split-fp8 x, split-fp8 W_qkv,
    # bf16 W_proj and bias rows
    def split8(a):
        hi = a.astype(F8NP)
        lo = (a - hi.astype(np.float32)).astype(F8NP)
        return np.ascontiguousarray(hi), np.ascontiguousarray(lo)

    w_h, w_l = split8(W_qkv * np.float32(1024.0))
    xt32 = np.ascontiguousarray(np.swapaxes(x, 1, 2)) * np.float32(32.0)
    xsplit = [split8(xt32[b]) for b in range(B)]
    wp_h, wp_l = split8(W_proj * np.float32(1024.0))
    wp45 = np.ascontiguousarray(
        (W_proj[512:, :] * np.float32(1.0 / 32.0)).astype(BF16NP))
    bv_b = np.ascontiguousarray(b_qkv[2 * E:].astype(BF16NP))
    bp_b = np.ascontiguousarray(b_proj.astype(BF16NP))

    in_maps = [
        {"xh": xsplit[b][0], "xl": xsplit[b][1],
         "W_qkvh": w_h, "W_qkvl": w_l, "b_qkv": b_qkv,
         "b_v": bv_b, "W_projh": wp_h, "W_projl": wp_l, "W_pb45": wp45,
         "b_pb": bp_b}
        for b in range(B)
    ]
    res = run_bass_kernel_spmd(nc, in_maps, core_ids=list(range(B)))
    return np.stack([np.asarray(res.results[b]["out"]) for b in range(B)])


# revision 44
# speedup vs baseline: 1.0015x; 1.0003x over previous
"""Multi-head attention block on 8 Trainium2 NeuronCores.

Problem: B=8, N=1024, E=768, H=12, D=64 attention (QKV proj -> softmax(QK^T/8)V
-> output proj), fp32 I/O. Data parallel over batch: core b owns batch b.

v4 design (split-fp8 DoubleRow QKV + all-bf16 attention, host preprocessing):
  - Host precomputes transposed split-fp8 x (xh+xl ~= 32*x^T) and split-fp8
    W_qkv (wh+wl ~= 1024*W_qkv), bf16 W_proj / bias rows. All device loads are
    plain HWDGE DMAs (no casts, no SWDGE descriptor generation, no PE
    transposes of x).
  - QKV projection: 9 fp8 DoubleRow matmuls per psum tile ((xh+xl)@(wh+wl)
    with the xl@wl term dropped), 256-deep contraction pairs at 0.5 cyc/row.
    Psum carries 2^15 scale; Q/K evac rescales (tensor_scalar mult+add bias),
    V keeps the scale which cancels against the 2^15 ones-column in Z.
  - S^T[k,q] per head: two 512-wide bf16 matmuls into a [128,1024] psum
    (contraction d=64 at partition base (h%2)*64); exp on Act -> bf16 expS.
  - U restructured: stationary = expS chunk [128k,128q], moving = V [128k,65]
    (64 dims + 2^15 ones column) -> psum U^T[q,65] accumulated over k chunks;
    invZ = reciprocal of column 64 is a per-partition scalar; attn = U*invZ
    is one DVE tensor_scalar op. Halves U's PE rows vs the classic layout
    and kills the PE invZ broadcast.
  - attn rows (token-major) -> attnT (feature-major) via HWDGE XBAR DMA
    transposes (3 [128,128] bf16 blocks per DMA, zero PE cost).
  - Output proj split: attnT blocks 0..2 projected during late attention as
    PE filler; blocks 3..5 in the tail, accumulated into the same SBUF tile.
  - Emission interleaves S psum fills with QK/V/U/proj filler units so the
    Act engine (exp is ~100us of work, the secondary wall) starves as little
    as possible while PE (the primary wall) stays busy.
"""
import numpy as np

B, N, E, H, D = 8, 1024, 768, 12, 64
SCALE = D ** -0.5
NT = N // 128   # token chunks (8)
NE = E // 128   # embed chunks (6)
NQ = N // 512   # moving-dim tiles (2)
NFS = [(0, 512), (512, 256)]  # free-dim split of E for matmuls
PROJ_SPLIT = 3  # attnT blocks 0..2 in projA (during attention), 3..5 in tail


def _build():
    import concourse.bacc as bacc
    import concourse.mybir as mybir
    import concourse.tile as tile

    F32 = mybir.dt.float32
    BF16 = mybir.dt.bfloat16
    F8 = mybir.dt.float8e4
    EXP = mybir.ActivationFunctionType.Exp
    DR = mybir.MatmulPerfMode.DoubleRow
    MUL = mybir.AluOpType.mult
    ADD = mybir.AluOpType.add

    nc = bacc.Bacc("TRN2", target_bir_lowering=False)
    xh_d = nc.declare_dram_parameter("xh", [E, N], F8, isOutput=False)
    xl_d = nc.declare_dram_parameter("xl", [E, N], F8, isOutput=False)
    wqkvh_d = nc.declare_dram_parameter("W_qkvh", [E, 3 * E], F8, isOutput=False)
    wqkvl_d = nc.declare_dram_parameter("W_qkvl", [E, 3 * E], F8, isOutput=False)
    bqkv_d = nc.declare_dram_parameter("b_qkv", [3 * E], F32, isOutput=False)
    bv_d = nc.declare_dram_parameter("b_v", [E], BF16, isOutput=False)
    wprojh_d = nc.declare_dram_parameter("W_projh", [E, E], F8, isOutput=False)
    wprojl_d = nc.declare_dram_parameter("W_projl", [E, E], F8, isOutput=False)
    wpb45_d = nc.declare_dram_parameter("W_pb45", [256, E], BF16, isOutput=False)
    bp_d = nc.declare_dram_parameter("b_pb", [E], BF16, isOutput=False)
    out_d = nc.declare_dram_parameter("out", [N, E], F32, isOutput=True)

    with tile.TileContext(nc) as tc:
        with (
            tc.tile_pool(name="const", bufs=1) as cp,
            tc.tile_pool(name="main", bufs=1) as qp,
            tc.tile_pool(name="psum", bufs=1, space="PSUM") as ps,
        ):
            # ---- constants ----
            ones1 = cp.tile([1, 128], BF16)
            nc.vector.memset(ones1, 1.0)
            ones32k = cp.tile([1, 128], BF16)
            nc.vector.memset(ones32k, 32768.0)
            bqc = cp.tile([128, 12], F32)   # column fc = b_qkv[128fc:128(fc+1)]

            # ---- long-lived tensors ----
            qT = [qp.tile([128, N], BF16, name=f"qT{c}", tag=f"qT{c}")
                  for c in range(6)]
            kT = [qp.tile([128, N], BF16, name=f"kT{c}", tag=f"kT{c}")
                  for c in range(6)]
            vS = [qp.tile([128, 65 * H], BF16, name=f"vS{i}", tag=f"vS{i}")
                  for i in range(NT)]
            attnS = [qp.tile([128, E], BF16, name=f"atS{i}", tag=f"atS{i}")
                     for i in range(NT)]
            attnT = qp.tile([128, NE * N], BF16)  # [128, (c, 1024)] = 32*attn^T
            attnTv = attnT.rearrange("p (c n) -> p c n", n=N)
            attnTh = qp.tile([128, NE * N], F8)
            attnThv = attnTh.rearrange("p (c n) -> p c n", n=N)
            attnTl = qp.tile([128, NE * N], F8)
            attnTlv = attnTl.rearrange("p (c n) -> p c n", n=N)
            # W_proj pair tiles: pair pp = rows 256pp..256pp+256 as [128, 2, E]
            wphB = qp.tile([128, 2 * 2 * E], F8)
            wplB = qp.tile([128, 2 * 2 * E], F8)
            wphv = wphB.rearrange("p (pp t f) -> p pp t f", t=2, f=E)
            wplv = wplB.rearrange("p (pp t f) -> p pp t f", t=2, f=E)
            wpb45 = qp.tile([128, 2 * E], BF16)
            wpb45v = wpb45.rearrange("p (c f) -> p c f", f=E)
            bv_bc = qp.tile([128, E], F32)
            bp_bc = qp.tile([128, E], F32)
            bv_row = qp.tile([1, E], BF16)
            bp_row = qp.tile([1, E], BF16)
            o_acc = [qp.tile([128, E], F32, name=f"oa{i}", tag=f"oa{i}")
                     for i in range(NT)]

            # expS pool: [128, N] bf16 tiles; 3 heads alive (lag 2)
            ep = tc.alloc_tile_pool(name="exp", bufs=1)
            iz = tc.alloc_tile_pool(name="iz", bufs=1)

            # scoped pool: x / W_qkv fp8 tiles, released once QKV is done
            xp = tc.alloc_tile_pool(name="xw", bufs=1)
            xH = xp.tile([128, NE * N], F8)   # [128, (j, 1024 tok)] = 32*x^T
            xL = xp.tile([128, NE * N], F8)
            xHv = xH.rearrange("p (j n) -> p j n", n=N)
            xLv = xL.rearrange("p (j n) -> p j n", n=N)
            # weight pair big tiles: [128, (p, t, f)] with pair p = W rows
            # 256p..256p+256 split as 2 k-subtiles t
            wqkhB = xp.tile([128, 3 * 2 * 1536], F8)
            wqklB = xp.tile([128, 3 * 2 * 1536], F8)
            wqkhv = wqkhB.rearrange("p (pp t f) -> p pp t f", t=2, f=1536)
            wqklv = wqklB.rearrange("p (pp t f) -> p pp t f", t=2, f=1536)
            wvhB = xp.tile([128, 3 * 2 * E], F8)
            wvlB = xp.tile([128, 3 * 2 * E], F8)
            wvhv = wvhB.rearrange("p (pp t f) -> p pp t f", t=2, f=E)
            wvlv = wvlB.rearrange("p (pp t f) -> p pp t f", t=2, f=E)

            # ---- DMAs (all HWDGE, no casts): few big transfers, with two
            # small priority slices so the first S unit starts early ----
            def wqk_slice(wview, w_d, c0, cw):
                nc.sync.dma_start(
                    out=wview[:, :, :, c0:c0 + cw],
                    in_=w_d[0:768, c0:c0 + cw].rearrange(
                        "(pp t k) f -> k pp t f", t=2, k=128))

            # 1. t=0 / t=6 weight columns (gate the first S unit)
            wqk_slice(wqkhv, wqkvh_d, 0, 128)
            wqk_slice(wqkhv, wqkvh_d, 768, 128)
            wqk_slice(wqklv, wqkvl_d, 0, 128)
            wqk_slice(wqklv, wqkvl_d, 768, 128)
            nc.sync.dma_start(
                out=bqc, in_=bqkv_d[0:1536].rearrange("(f p) -> p f", p=128))
            # 2. x token-half 0, then half 1
            for half in range(2):
                t0 = half * 512
                nc.sync.dma_start(
                    out=xHv[:, :, t0:t0 + 512],
                    in_=xh_d[:, t0:t0 + 512].rearrange(
                        "(j k) n -> k j n", k=128))
                nc.sync.dma_start(
                    out=xLv[:, :, t0:t0 + 512],
                    in_=xl_d[:, t0:t0 + 512].rearrange(
                        "(j k) n -> k j n", k=128))
            nc.sync.dma_start(
                out=bv_row, in_=bv_d[:].rearrange("(o f) -> o f", o=1))
            nc.sync.dma_start(
                out=bp_row, in_=bp_d[:].rearrange("(o f) -> o f", o=1))
            # 3. remaining wqk columns
            wqk_slice(wqkhv, wqkvh_d, 128, 640)
            wqk_slice(wqklv, wqkvl_d, 128, 640)
            wqk_slice(wqkhv, wqkvh_d, 896, 640)
            wqk_slice(wqklv, wqkvl_d, 896, 640)
            # 4. V weights, bias rows, proj weights
            for w_t, w_d in ((wvhB, wqkvh_d), (wvlB, wqkvl_d)):
                nc.sync.dma_start(
                    out=w_t.rearrange("p (pp t f) -> p pp t f", t=2, f=E),
                    in_=w_d[0:768, 1536:].rearrange(
                        "(pp t k) f -> k pp t f", t=2, k=128))
            for w_t, w_d in ((wphB, wprojh_d), (wplB, wprojl_d)):
                nc.sync.dma_start(
                    out=w_t.rearrange("p (pp t f) -> p pp t f", t=2, f=E),
                    in_=w_d[0:512, :].rearrange(
                        "(pp t k) f -> k pp t f", t=2, k=128))
            nc.sync.dma_start(
                out=wpb45.rearrange("p (c f) -> p c f", f=E),
                in_=wpb45_d[:].rearrange("(c k) f -> k c f", k=128))

            def emit_prelude():
                for nf, (f0, fw) in enumerate(NFS):
                    pbv = ps.tile([128, 512], F32, name=f"pbv{nf}", tag="mm",
                                  bufs=2)
                    nc.tensor.matmul(pbv[:, :fw], ones32k,
                                     bv_row[:, f0:f0 + fw],
                                     start=True, stop=True)
                    nc.vector.tensor_copy(bv_bc[:, f0:f0 + fw], pbv[:, :fw])
                    pbp = ps.tile([128, 512], F32, name=f"pbp{nf}", tag="mm",
                                  bufs=2)
                    nc.tensor.matmul(pbp[:, :fw], ones32k,
                                     bp_row[:, f0:f0 + fw],
                                     start=True, stop=True)
                    nc.vector.tensor_copy(bp_bc[:, f0:f0 + fw], pbp[:, :fw])
                for i in range(NT):
                    nc.vector.memset(
                        vS[i].rearrange("p (h c) -> p h c", c=65)[:, :, 64:65],
                        1024.0)

            # ================= emission units =================
            def emit_qk_unit(t, q, evac_act=False):
                """One (feature-tile, 512-token-half) of Q or K projection.
                Split-fp8 DoubleRow: (xh+xl)@(wh+wl), xl@wl dropped."""
                dst = qT[t] if t < 6 else kT[t - 6]
                wcol0 = t * 128
                pq = ps.tile([128, 512], F32, name=f"pq{t}_{q}", tag="mm",
                             bufs=2)
                terms = [(wqkhv, xHv), (wqkhv, xLv), (wqklv, xHv)]
                nmm = 3 * len(terms)
                mi = 0
                for p in range(3):
                    for w_v, x_v in terms:
                        nc.tensor.matmul(
                            pq,
                            w_v[:, p, :, wcol0:wcol0 + 128],
                            x_v[:, 2 * p:2 * p + 2, q * 512:(q + 1) * 512],
                            start=(mi == 0), stop=(mi == nmm - 1),
                            perf_mode=DR)
                        mi += 1
                if evac_act:
                    nc.scalar.activation(
                        dst[:, q * 512:(q + 1) * 512], pq,
                        mybir.ActivationFunctionType.Identity,
                        bias=bqc[:, t:t + 1], scale=float(2.0 ** -15))
                else:
                    nc.vector.tensor_scalar(
                        out=dst[:, q * 512:(q + 1) * 512], in0=pq,
                        scalar1=float(2.0 ** -15), scalar2=bqc[:, t:t + 1],
                        op0=MUL, op1=ADD)

            def emit_v_unit(i, nf):
                """One (token-chunk, free-half) of the V projection."""
                f0, fw = NFS[nf]
                pv = ps.tile([128, 512], F32, name=f"pv{i}_{nf}", tag="mm",
                             bufs=2)
                terms = [(xHv, wvhv), (xHv, wvlv), (xLv, wvhv)]
                nmm = 3 * len(terms)
                mi = 0
                for p in range(3):
                    for x_v, w_v in terms:
                        nc.tensor.matmul(
                            pv[:, :fw],
                            x_v[:, 2 * p:2 * p + 2, i * 128:(i + 1) * 128],
                            w_v[:, p, :, f0:f0 + fw],
                            start=(mi == 0), stop=(mi == nmm - 1),
                            perf_mode=DR)
                        mi += 1
                nh, h0 = fw // D, f0 // D
                nc.vector.tensor_add(
                    vS[i].rearrange("p (h c) -> p h c", c=65)
                        [:, h0:h0 + nh, 0:64],
                    pv[:, :fw].rearrange("p (h d) -> p h d", d=D),
                    bv_bc[:, f0:f0 + fw].rearrange("p (h d) -> p h d", d=D))

            expS_of = {}

            def emit_s_unit(h, kc):
                """S^T[k-chunk, all q] for one head + exp -> bf16 expS."""
                c, r0 = h // 2, (h % 2) * 64
                if kc == 0:
                    expS_of[h] = [
                        ep.tile([128, N], BF16, name=f"eS{h}_{k2}",
                                tag="expS", bufs=24)
                        for k2 in range(NT)]
                pss = ps.tile([128, N], F32, name=f"ps{h}_{kc}", tag="s2",
                              bufs=2)
                for q in range(NQ):
                    nc.tensor.matmul(
                        pss[:, q * 512:(q + 1) * 512],
                        kT[c][r0:r0 + 64, kc * 128:(kc + 1) * 128],
                        qT[c][r0:r0 + 64, q * 512:(q + 1) * 512],
                        start=True, stop=True)
                    if h == 0 and kc == 0:
                        nc.scalar.activation(
                            expS_of[h][kc][:, q * 512:(q + 1) * 512],
                            pss[:, q * 512:(q + 1) * 512], EXP,
                            scale=float(SCALE))
                if not (h == 0 and kc == 0):
                    nc.scalar.activation(expS_of[h][kc], pss, EXP,
                                         scale=float(SCALE))

            def emit_u_unit(h, qc, evac_act=False):
                """U^T[q-chunk, 65] for one head; normalize into attnS."""
                expS = expS_of[h]
                pu = ps.tile([128, 512], F32, name=f"pu{h}_{qc}", tag="u",
                             bufs=2)
                for kc in range(NT):
                    nc.tensor.matmul(
                        pu[:, 0:65],
                        expS[kc][:, qc * 128:(qc + 1) * 128],
                        vS[kc][:, h * 65:h * 65 + 65],
                        start=(kc == 0), stop=(kc == NT - 1))
                invz = iz.tile([128, 1], F32, name=f"iv{h}_{qc}", tag="iz",
                               bufs=3)
                nc.vector.reciprocal(invz, pu[:, 64:65])
                if evac_act:
                    nc.scalar.activation(
                        attnS[qc][:, h * D:(h + 1) * D], pu[:, 0:64],
                        mybir.ActivationFunctionType.Copy, scale=invz)
                else:
                    nc.vector.tensor_scalar_mul(
                        attnS[qc][:, h * D:(h + 1) * D], pu[:, 0:64], invz)

            def emit_transp(qc, pp):
                """DMA-transpose one block pair into attnT (32*attn^T bf16),
                then split to fp8 hi/lo on the Pool engine."""
                c0 = 2 * pp
                t_bf = attnTv[:, c0:c0 + 2, qc * 128:(qc + 1) * 128]
                nc.sync.dma_start_transpose(
                    t_bf, attnS[qc][:, c0 * 128:(c0 + 2) * 128])
                if pp < 2:
                    t_hi = attnThv[:, c0:c0 + 2, qc * 128:(qc + 1) * 128]
                    nc.gpsimd.tensor_copy(t_hi, t_bf)
                    nc.gpsimd.tensor_sub(
                        attnTlv[:, c0:c0 + 2, qc * 128:(qc + 1) * 128],
                        t_bf, t_hi)

            def emit_proj_unit(i, nf, pp):
                """Half-row of output proj for block pair pp (split-fp8 DR).
                pp=0 adds the (2^15-scaled) bias; pp=1 accumulates; pp=2
                accumulates and rescales to the final f32 output."""
                f0, fw = NFS[nf]
                po = ps.tile([128, 512], F32, name=f"po{i}_{nf}_{pp}",
                             tag="mm", bufs=2)
                if pp < 2:
                    terms = [(attnThv, wphv), (attnThv, wplv), (attnTlv, wphv)]
                    for mi, (a_v, w_v) in enumerate(terms):
                        nc.tensor.matmul(
                            po[:, :fw],
                            a_v[:, 2 * pp:2 * pp + 2, i * 128:(i + 1) * 128],
                            w_v[:, pp, :, f0:f0 + fw],
                            start=(mi == 0), stop=(mi == len(terms) - 1),
                            perf_mode=DR)
                else:
                    for ci, cb in enumerate((4, 5)):
                        nc.tensor.matmul(
                            po[:, :fw],
                            attnTv[:, cb, i * 128:(i + 1) * 128],
                            wpb45v[:, cb - 4, f0:f0 + fw],
                            start=(ci == 0), stop=(ci == 1))
                if pp == 0:
                    nc.vector.tensor_add(
                        o_acc[i][:, f0:f0 + fw], po[:, :fw],
                        bp_bc[:, f0:f0 + fw])
                elif pp == 1:
                    nc.vector.tensor_add(
                        o_acc[i][:, f0:f0 + fw], po[:, :fw],
                        o_acc[i][:, f0:f0 + fw])
                    # rescale the pairs-0/1(+bias) partial here, off the tail
                    nc.gpsimd.tensor_scalar_mul(
                        o_acc[i][:, f0:f0 + fw], o_acc[i][:, f0:f0 + fw],
                        float(2.0 ** -15))
                else:
                    nc.vector.tensor_add(
                        o_acc[i][:, f0:f0 + fw], po[:, :fw],
                        o_acc[i][:, f0:f0 + fw])

            # ================= schedule =================
            for q in range(NQ):
                for t in (0, 6):
                    emit_qk_unit(t, q)

            for h in range(H):
                c = h // 2
                fillers = []
                if h == 0:
                    fillers.append(("prelude",))
                if h < 10:  # QK chunk c+1: 2 units during each of h=2c, 2c+1
                    t0 = (c + 1, 6 + c + 1)
                    if h % 2 == 0:
                        fillers.append(("qk", t0[0], 0))
                        fillers.append(("qk", t0[1], 0))
                    else:
                        fillers.append(("qk", t0[0], 1))
                        fillers.append(("qk", t0[1], 1))
                if h in (1, 2):  # V projection (wv lands early now)
                    lo, hi = (0, 3) if h == 1 else (3, 8)
                    for i in range(lo, hi):
                        fillers.append(("v", i, 0))
                        fillers.append(("v", i, 1))
                if h >= 2:  # U for head h-2 (after V units at h=2)
                    for qc in range(NT):
                        fillers.append(("u", h - 2, qc))
                if h == 10:
                    fillers.append(("xfree",))
                if h in (6, 7):  # proj pair 0 (blocks 0,1; ready after U(3))
                    for i in range(4 * (h - 6), 4 * (h - 6) + 4):
                        fillers.append(("pa", i, 0, 0))
                        fillers.append(("pa", i, 1, 0))
                if h in (10, 11):  # proj pair 1 (blocks 2,3; after U(7))
                    for i in range(4 * (h - 10), 4 * (h - 10) + 4):
                        fillers.append(("pa", i, 0, 1))
                        fillers.append(("pa", i, 1, 1))
                if h == 11:  # U(10) late in head 11 (exp(10) done by then)
                    for qc in range(NT):
                        fillers.append(("u", 10, qc))

                fillers.sort(
                    key=lambda f: (f[0] == "u", f[0] == "prelude"))

                def drain(k):
                    for _ in range(k):
                        if not fillers:
                            return
                        f = fillers.pop(0)
                        if f[0] == "v":
                            emit_v_unit(f[1], f[2])
                        elif f[0] == "qk":
                            emit_qk_unit(f[1], f[2])
                        elif f[0] == "u":
                            emit_u_unit(f[1], f[2])
                            if f[1] == 3:
                                emit_transp(f[2], 0)
                            elif f[1] == 7:
                                emit_transp(f[2], 1)
                        elif f[0] == "pa":
                            emit_proj_unit(f[1], f[2], f[3])
                        elif f[0] == "xfree":
                            xp.release()
                        elif f[0] == "prelude":
                            emit_prelude()

                for kc in range(NT):
                    emit_s_unit(h, kc)
                    drain((len(fillers) + NT - kc - 1) // (NT - kc))
                drain(len(fillers))

            # ---- tail: U(11) + attnT blocks 4,5 + proj pair 2, pipelined
            def emit_out(i):
                po = ps.tile([128, 1024], F32, name=f"pot{i}", tag="s2",
                             bufs=2)
                for nf, (f0, fw) in enumerate(NFS):
                    for ci, cb in enumerate((4, 5)):
                        nc.tensor.matmul(
                            po[:, f0:f0 + fw],
                            attnTv[:, cb, i * 128:(i + 1) * 128],
                            wpb45v[:, cb - 4, f0:f0 + fw],
                            start=(ci == 0), stop=(ci == 1))
                nc.vector.tensor_add(
                    o_acc[i], po[:, 0:E], o_acc[i])
                nc.sync.dma_start(
                    out=out_d[i * 128:(i + 1) * 128, :], in_=o_acc[i])

            for qc in range(NT):
                emit_u_unit(11, qc)
                emit_transp(qc, 2)
            for qc in range(NT):
                emit_out(qc)
            iz.release()
            ep.release()
    nc.compile()
    return nc


_NC_CACHE = None


def kernel(x, W_qkv, b_qkv, W_proj, b_proj):
    from concourse.bass_utils import run_bass_kernel_spmd
    import ml_dtypes

    F8NP = ml_dtypes.float8_e4m3
    BF16NP = ml_dtypes.bfloat16

    global _NC_CACHE
    if _NC_CACHE is None:
        _NC_CACHE = _build()
    nc = _NC_CACHE

    x = np.asarray(x, dtype=np.float32)
    W_qkv = np.asarray(W_qkv, dtype=np.float32)
    b_qkv = np.ascontiguousarray(np.asarray(b_qkv, dtype=np.float32))
    W_proj = np.asarray(W_proj, dtype=np.float32)
    b_proj = np.asarray(b_proj, dtype=np.float32)

    # host-side preprocessing: transposed split-fp8 x, split-fp8 W_qkv,
    # bf16 W_proj and bias rows
    def split8(a):
        hi = a.astype(F8NP)
        lo = (a - hi.astype(np.float32)).astype(F8NP)
        return np.ascontiguousarray(hi), np.ascontiguousarray(lo)

    w_h, w_l = split8(W_qkv * np.float32(1024.0))
    xt32 = np.ascontiguousarray(np.swapaxes(x, 1, 2)) * np.float32(32.0)
    xsplit = [split8(xt32[b]) for b in range(B)]
    wp_h, wp_l = split8(W_proj * np.float32(1024.0))
    wp45 = np.ascontiguousarray(
        (W_proj[512:, :] * np.float32(1.0 / 32.0)).astype(BF16NP))
    bv_b = np.ascontiguousarray(b_qkv[2 * E:].astype(BF16NP))
    bp_b = np.ascontiguousarray(b_proj.astype(BF16NP))

    in_maps = [
        {"xh": xsplit[b][0], "xl": xsplit[b][1],
         "W_qkvh": w_h, "W_qkvl": w_l, "b_qkv": b_qkv,
         "b_v": bv_b, "W_projh": wp_h, "W_projl": wp_l, "W_pb45": wp45,
         "b_pb": bp_b}
        for b in range(B)
    ]
    res = run_bass_kernel_spmd(nc, in_maps, core_ids=list(range(B)))
    return np.stack([np.asarray(res.results[b]["out"]) for b in range(B)])


# revision 49
# speedup vs baseline: 1.0085x; 1.0070x over previous
"""Multi-head attention block on 8 Trainium2 NeuronCores.

Problem: B=8, N=1024, E=768, H=12, D=64 attention (QKV proj -> softmax(QK^T/8)V
-> output proj), fp32 I/O. Data parallel over batch: core b owns batch b.

v4 design (split-fp8 DoubleRow QKV + all-bf16 attention, host preprocessing):
  - Host precomputes transposed split-fp8 x (xh+xl ~= 32*x^T) and split-fp8
    W_qkv (wh+wl ~= 1024*W_qkv), bf16 W_proj / bias rows. All device loads are
    plain HWDGE DMAs (no casts, no SWDGE descriptor generation, no PE
    transposes of x).
  - QKV projection: 9 fp8 DoubleRow matmuls per psum tile ((xh+xl)@(wh+wl)
    with the xl@wl term dropped), 256-deep contraction pairs at 0.5 cyc/row.
    Psum carries 2^15 scale; Q/K evac rescales (tensor_scalar mult+add bias),
    V keeps the scale which cancels against the 2^15 ones-column in Z.
  - S^T[k,q] per head: two 512-wide bf16 matmuls into a [128,1024] psum
    (contraction d=64 at partition base (h%2)*64); exp on Act -> bf16 expS.
  - U restructured: stationary = expS chunk [128k,128q], moving = V [128k,65]
    (64 dims + 2^15 ones column) -> psum U^T[q,65] accumulated over k chunks;
    invZ = reciprocal of column 64 is a per-partition scalar; attn = U*invZ
    is one DVE tensor_scalar op. Halves U's PE rows vs the classic layout
    and kills the PE invZ broadcast.
  - attn rows (token-major) -> attnT (feature-major) via HWDGE XBAR DMA
    transposes (3 [128,128] bf16 blocks per DMA, zero PE cost).
  - Output proj split: attnT blocks 0..2 projected during late attention as
    PE filler; blocks 3..5 in the tail, accumulated into the same SBUF tile.
  - Emission interleaves S psum fills with QK/V/U/proj filler units so the
    Act engine (exp is ~100us of work, the secondary wall) starves as little
    as possible while PE (the primary wall) stays busy.
"""
import numpy as np

B, N, E, H, D = 8, 1024, 768, 12, 64
SCALE = D ** -0.5
NT = N // 128   # token chunks (8)
NE = E // 128   # embed chunks (6)
NQ = N // 512   # moving-dim tiles (2)
NFS = [(0, 512), (512, 256)]  # free-dim split of E for matmuls
PROJ_SPLIT = 3  # attnT blocks 0..2 in projA (during attention), 3..5 in tail


def _build():
    import concourse.bacc as bacc
    import concourse.mybir as mybir
    import concourse.tile as tile

    F32 = mybir.dt.float32
    BF16 = mybir.dt.bfloat16
    F8 = mybir.dt.float8e4
    EXP = mybir.ActivationFunctionType.Exp
    DR = mybir.MatmulPerfMode.DoubleRow
    MUL = mybir.AluOpType.mult
    ADD = mybir.AluOpType.add

    nc = bacc.Bacc("TRN2", target_bir_lowering=False)
    xh_d = nc.declare_dram_parameter("xh", [E, N], F8, isOutput=False)
    xl_d = nc.declare_dram_parameter("xl", [E, N], F8, isOutput=False)
    wqkvh_d = nc.declare_dram_parameter("W_qkvh", [E, 3 * E], F8, isOutput=False)
    wqkvl_d = nc.declare_dram_parameter("W_qkvl", [E, 3 * E], F8, isOutput=False)
    wfh_d = nc.declare_dram_parameter("W_fh", [E, 256], F8, isOutput=False)
    wfl_d = nc.declare_dram_parameter("W_fl", [E, 256], F8, isOutput=False)
    bqkv_d = nc.declare_dram_parameter("b_qkv", [3 * E], F32, isOutput=False)
    bv_d = nc.declare_dram_parameter("b_v", [E], BF16, isOutput=False)
    wprojh_d = nc.declare_dram_parameter("W_projh", [E, E], F8, isOutput=False)
    wprojl_d = nc.declare_dram_parameter("W_projl", [E, E], F8, isOutput=False)
    wpb45_d = nc.declare_dram_parameter("W_pb45", [256, E], BF16, isOutput=False)
    bp_d = nc.declare_dram_parameter("b_pb", [E], BF16, isOutput=False)
    out_d = nc.declare_dram_parameter("out", [N, E], F32, isOutput=True)

    with tile.TileContext(nc) as tc:
        with (
            tc.tile_pool(name="const", bufs=1) as cp,
            tc.tile_pool(name="main", bufs=1) as qp,
            tc.tile_pool(name="psum", bufs=1, space="PSUM") as ps,
        ):
            # ---- constants ----
            ones1 = cp.tile([1, 128], BF16)
            nc.vector.memset(ones1, 1.0)
            ones32k = cp.tile([1, 128], BF16)
            nc.vector.memset(ones32k, 32768.0)
            bqc = cp.tile([128, 12], F32)   # column fc = b_qkv[128fc:128(fc+1)]

            # ---- long-lived tensors ----
            qT = [qp.tile([128, N], BF16, name=f"qT{c}", tag=f"qT{c}")
                  for c in range(6)]
            kT = [qp.tile([128, N], BF16, name=f"kT{c}", tag=f"kT{c}")
                  for c in range(6)]
            vS = [qp.tile([128, 65 * H], BF16, name=f"vS{i}", tag=f"vS{i}")
                  for i in range(NT)]
            attnS = [qp.tile([128, E], BF16, name=f"atS{i}", tag=f"atS{i}")
                     for i in range(NT)]
            attnT = qp.tile([128, NE * N], BF16)  # [128, (c, 1024)] = 32*attn^T
            attnTv = attnT.rearrange("p (c n) -> p c n", n=N)
            attnTh = qp.tile([128, NE * N], F8)
            attnThv = attnTh.rearrange("p (c n) -> p c n", n=N)
            attnTl = qp.tile([128, NE * N], F8)
            attnTlv = attnTl.rearrange("p (c n) -> p c n", n=N)
            # W_proj pair tiles: pair pp = rows 256pp..256pp+256 as [128, 2, E]
            wphB = qp.tile([128, 2 * 2 * E], F8)
            wplB = qp.tile([128, 2 * 2 * E], F8)
            wphv = wphB.rearrange("p (pp t f) -> p pp t f", t=2, f=E)
            wplv = wplB.rearrange("p (pp t f) -> p pp t f", t=2, f=E)
            wpb45 = qp.tile([128, 2 * E], BF16)
            wpb45v = wpb45.rearrange("p (c f) -> p c f", f=E)
            bv_bc = qp.tile([128, E], F32)
            bp_bc = qp.tile([128, E], F32)
            bv_row = qp.tile([1, E], BF16)
            bp_row = qp.tile([1, E], BF16)
            o_acc = [qp.tile([128, E], F32, name=f"oa{i}", tag=f"oa{i}")
                     for i in range(NT)]

            # expS pool: [128, N] bf16 tiles; 3 heads alive (lag 2)
            ep = tc.alloc_tile_pool(name="exp", bufs=1)
            iz = tc.alloc_tile_pool(name="iz", bufs=1)

            # scoped pool: x / W_qkv fp8 tiles, released once QKV is done
            xp = tc.alloc_tile_pool(name="xw", bufs=1)
            xH = xp.tile([128, NE * N], F8)   # [128, (j, 1024 tok)] = 32*x^T
            xL = xp.tile([128, NE * N], F8)
            xHv = xH.rearrange("p (j n) -> p j n", n=N)
            xLv = xL.rearrange("p (j n) -> p j n", n=N)
            # weight pair big tiles: [128, (p, t, f)] with pair p = W rows
            # 256p..256p+256 split as 2 k-subtiles t
            wqkhB = xp.tile([128, 3 * 2 * 1536], F8)
            wqklB = xp.tile([128, 3 * 2 * 1536], F8)
            wqkhv = wqkhB.rearrange("p (pp t f) -> p pp t f", t=2, f=1536)
            wqklv = wqklB.rearrange("p (pp t f) -> p pp t f", t=2, f=1536)
            wqk0h = xp.tile([128, 3 * 2 * 256], F8)
            wqk0l = xp.tile([128, 3 * 2 * 256], F8)
            wqk0hv = wqk0h.rearrange("p (pp t f) -> p pp t f", t=2, f=256)
            wqk0lv = wqk0l.rearrange("p (pp t f) -> p pp t f", t=2, f=256)
            wvhB = xp.tile([128, 3 * 2 * E], F8)
            wvlB = xp.tile([128, 3 * 2 * E], F8)
            wvhv = wvhB.rearrange("p (pp t f) -> p pp t f", t=2, f=E)
            wvlv = wvlB.rearrange("p (pp t f) -> p pp t f", t=2, f=E)

            # ---- DMAs (all HWDGE, no casts): few big transfers, with two
            # small priority slices so the first S unit starts early ----
            def wqk_slice(wview, w_d, c0, cw):
                nc.sync.dma_start(
                    out=wview[:, :, :, c0:c0 + cw],
                    in_=w_d[0:768, c0:c0 + cw].rearrange(
                        "(pp t k) f -> k pp t f", t=2, k=128))

            # 1. host-packed t=0|t=6 weight columns (gate the first S unit)
            for w_t, w_d in ((wqk0h, wfh_d), (wqk0l, wfl_d)):
                nc.sync.dma_start(
                    out=w_t.rearrange("p (pp t f) -> p pp t f", t=2, f=256),
                    in_=w_d[:, :].rearrange(
                        "(pp t k) f -> k pp t f", t=2, k=128))
            # 2. x token-half 0, then half 1
            for half in range(2):
                t0 = half * 512
                nc.sync.dma_start(
                    out=xHv[:, :, t0:t0 + 512],
                    in_=xh_d[:, t0:t0 + 512].rearrange(
                        "(j k) n -> k j n", k=128))
                nc.sync.dma_start(
                    out=xLv[:, :, t0:t0 + 512],
                    in_=xl_d[:, t0:t0 + 512].rearrange(
                        "(j k) n -> k j n", k=128))
            nc.sync.dma_start(
                out=bqc, in_=bqkv_d[0:1536].rearrange("(f p) -> p f", p=128))
            nc.sync.dma_start(
                out=bv_row, in_=bv_d[:].rearrange("(o f) -> o f", o=1))
            nc.sync.dma_start(
                out=bp_row, in_=bp_d[:].rearrange("(o f) -> o f", o=1))
            # 3. remaining wqk columns
            wqk_slice(wqkhv, wqkvh_d, 128, 640)
            wqk_slice(wqklv, wqkvl_d, 128, 640)
            wqk_slice(wqkhv, wqkvh_d, 896, 640)
            wqk_slice(wqklv, wqkvl_d, 896, 640)
            # 4. V weights, bias rows, proj weights
            for w_t, w_d in ((wvhB, wqkvh_d), (wvlB, wqkvl_d)):
                nc.sync.dma_start(
                    out=w_t.rearrange("p (pp t f) -> p pp t f", t=2, f=E),
                    in_=w_d[0:768, 1536:].rearrange(
                        "(pp t k) f -> k pp t f", t=2, k=128))
            for w_t, w_d in ((wphB, wprojh_d), (wplB, wprojl_d)):
                nc.sync.dma_start(
                    out=w_t.rearrange("p (pp t f) -> p pp t f", t=2, f=E),
                    in_=w_d[0:512, :].rearrange(
                        "(pp t k) f -> k pp t f", t=2, k=128))
            nc.sync.dma_start(
                out=wpb45.rearrange("p (c f) -> p c f", f=E),
                in_=wpb45_d[:].rearrange("(c k) f -> k c f", k=128))

            def emit_prelude():
                for nf, (f0, fw) in enumerate(NFS):
                    pbv = ps.tile([128, 512], F32, name=f"pbv{nf}", tag="mm",
                                  bufs=2)
                    nc.tensor.matmul(pbv[:, :fw], ones32k,
                                     bv_row[:, f0:f0 + fw],
                                     start=True, stop=True)
                    nc.vector.tensor_copy(bv_bc[:, f0:f0 + fw], pbv[:, :fw])
                    pbp = ps.tile([128, 512], F32, name=f"pbp{nf}", tag="mm",
                                  bufs=2)
                    nc.tensor.matmul(pbp[:, :fw], ones32k,
                                     bp_row[:, f0:f0 + fw],
                                     start=True, stop=True)
                    nc.vector.tensor_copy(bp_bc[:, f0:f0 + fw], pbp[:, :fw])
                for i in range(NT):
                    nc.vector.memset(
                        vS[i].rearrange("p (h c) -> p h c", c=65)[:, :, 64:65],
                        1024.0)

            # ================= emission units =================
            def emit_qk_unit(t, q, evac_act=False, first=False):
                """One (feature-tile, 512-token-half) of Q or K projection.
                Split-fp8 DoubleRow: (xh+xl)@(wh+wl), xl@wl dropped."""
                dst = qT[t] if t < 6 else kT[t - 6]
                pq = ps.tile([128, 512], F32, name=f"pq{t}_{q}", tag="mm",
                             bufs=2)
                if first:
                    wcol0 = 0 if t == 0 else 128
                    terms = [(wqk0hv, xHv), (wqk0hv, xLv), (wqk0lv, xHv)]
                else:
                    wcol0 = t * 128
                    terms = [(wqkhv, xHv), (wqkhv, xLv), (wqklv, xHv)]
                nmm = 3 * len(terms)
                mi = 0
                for p in range(3):
                    for w_v, x_v in terms:
                        nc.tensor.matmul(
                            pq,
                            w_v[:, p, :, wcol0:wcol0 + 128],
                            x_v[:, 2 * p:2 * p + 2, q * 512:(q + 1) * 512],
                            start=(mi == 0), stop=(mi == nmm - 1),
                            perf_mode=DR)
                        mi += 1
                if evac_act:
                    nc.scalar.activation(
                        dst[:, q * 512:(q + 1) * 512], pq,
                        mybir.ActivationFunctionType.Identity,
                        bias=bqc[:, t:t + 1], scale=float(2.0 ** -15))
                else:
                    nc.vector.tensor_scalar(
                        out=dst[:, q * 512:(q + 1) * 512], in0=pq,
                        scalar1=float(2.0 ** -15), scalar2=bqc[:, t:t + 1],
                        op0=MUL, op1=ADD)

            def emit_v_unit(i, nf):
                """One (token-chunk, free-half) of the V projection."""
                f0, fw = NFS[nf]
                pv = ps.tile([128, 512], F32, name=f"pv{i}_{nf}", tag="mm",
                             bufs=2)
                terms = [(xHv, wvhv), (xHv, wvlv), (xLv, wvhv)]
                nmm = 3 * len(terms)
                mi = 0
                for p in range(3):
                    for x_v, w_v in terms:
                        nc.tensor.matmul(
                            pv[:, :fw],
                            x_v[:, 2 * p:2 * p + 2, i * 128:(i + 1) * 128],
                            w_v[:, p, :, f0:f0 + fw],
                            start=(mi == 0), stop=(mi == nmm - 1),
                            perf_mode=DR)
                        mi += 1
                nh, h0 = fw // D, f0 // D
                nc.vector.tensor_add(
                    vS[i].rearrange("p (h c) -> p h c", c=65)
                        [:, h0:h0 + nh, 0:64],
                    pv[:, :fw].rearrange("p (h d) -> p h d", d=D),
                    bv_bc[:, f0:f0 + fw].rearrange("p (h d) -> p h d", d=D))

            expS_of = {}

            def emit_s_unit(h, kc):
                """S^T[k-chunk, all q] for one head + exp -> bf16 expS."""
                c, r0 = h // 2, (h % 2) * 64
                if kc == 0:
                    expS_of[h] = [
                        ep.tile([128, N], BF16, name=f"eS{h}_{k2}",
                                tag="expS", bufs=24)
                        for k2 in range(NT)]
                pss = ps.tile([128, N], F32, name=f"ps{h}_{kc}", tag="s2",
                              bufs=2)
                for q in range(NQ):
                    nc.tensor.matmul(
                        pss[:, q * 512:(q + 1) * 512],
                        kT[c][r0:r0 + 64, kc * 128:(kc + 1) * 128],
                        qT[c][r0:r0 + 64, q * 512:(q + 1) * 512],
                        start=True, stop=True)
                    if h == 0 and kc == 0:
                        nc.scalar.activation(
                            expS_of[h][kc][:, q * 512:(q + 1) * 512],
                            pss[:, q * 512:(q + 1) * 512], EXP,
                            scale=float(SCALE))
                if not (h == 0 and kc == 0):
                    nc.scalar.activation(expS_of[h][kc], pss, EXP,
                                         scale=float(SCALE))

            def emit_u_unit(h, qc, evac_act=False):
                """U^T[q-chunk, 65] for one head; normalize into attnS."""
                expS = expS_of[h]
                pu = ps.tile([128, 512], F32, name=f"pu{h}_{qc}", tag="u",
                             bufs=2)
                for kc in range(NT):
                    nc.tensor.matmul(
                        pu[:, 0:65],
                        expS[kc][:, qc * 128:(qc + 1) * 128],
                        vS[kc][:, h * 65:h * 65 + 65],
                        start=(kc == 0), stop=(kc == NT - 1))
                invz = iz.tile([128, 1], F32, name=f"iv{h}_{qc}", tag="iz",
                               bufs=3)
                nc.vector.reciprocal(invz, pu[:, 64:65])
                if evac_act:
                    nc.scalar.activation(
                        attnS[qc][:, h * D:(h + 1) * D], pu[:, 0:64],
                        mybir.ActivationFunctionType.Copy, scale=invz)
                else:
                    nc.vector.tensor_scalar_mul(
                        attnS[qc][:, h * D:(h + 1) * D], pu[:, 0:64], invz)

            def emit_transp(qc, pp):
                """DMA-transpose one block pair into attnT (32*attn^T bf16),
                then split to fp8 hi/lo on the Pool engine."""
                c0 = 2 * pp
                t_bf = attnTv[:, c0:c0 + 2, qc * 128:(qc + 1) * 128]
                nc.sync.dma_start_transpose(
                    t_bf, attnS[qc][:, c0 * 128:(c0 + 2) * 128])
                if pp < 2:
                    t_hi = attnThv[:, c0:c0 + 2, qc * 128:(qc + 1) * 128]
                    nc.gpsimd.tensor_copy(t_hi, t_bf)
                    nc.gpsimd.tensor_sub(
                        attnTlv[:, c0:c0 + 2, qc * 128:(qc + 1) * 128],
                        t_bf, t_hi)

            def emit_proj_unit(i, nf, pp):
                """Half-row of output proj for block pair pp (split-fp8 DR).
                pp=0 adds the (2^15-scaled) bias; pp=1 accumulates; pp=2
                accumulates and rescales to the final f32 output."""
                f0, fw = NFS[nf]
                po = ps.tile([128, 512], F32, name=f"po{i}_{nf}_{pp}",
                             tag="mm", bufs=2)
                if pp < 2:
                    terms = [(attnThv, wphv), (attnThv, wplv), (attnTlv, wphv)]
                    for mi, (a_v, w_v) in enumerate(terms):
                        nc.tensor.matmul(
                            po[:, :fw],
                            a_v[:, 2 * pp:2 * pp + 2, i * 128:(i + 1) * 128],
                            w_v[:, pp, :, f0:f0 + fw],
                            start=(mi == 0), stop=(mi == len(terms) - 1),
                            perf_mode=DR)
                else:
                    for ci, cb in enumerate((4, 5)):
                        nc.tensor.matmul(
                            po[:, :fw],
                            attnTv[:, cb, i * 128:(i + 1) * 128],
                            wpb45v[:, cb - 4, f0:f0 + fw],
                            start=(ci == 0), stop=(ci == 1))
                if pp == 0:
                    nc.vector.tensor_add(
                        o_acc[i][:, f0:f0 + fw], po[:, :fw],
                        bp_bc[:, f0:f0 + fw])
                elif pp == 1:
                    nc.vector.tensor_add(
                        o_acc[i][:, f0:f0 + fw], po[:, :fw],
                        o_acc[i][:, f0:f0 + fw])
                    # rescale the pairs-0/1(+bias) partial here, off the tail
                    nc.gpsimd.tensor_scalar_mul(
                        o_acc[i][:, f0:f0 + fw], o_acc[i][:, f0:f0 + fw],
                        float(2.0 ** -15))
                else:
                    nc.vector.tensor_add(
                        o_acc[i][:, f0:f0 + fw], po[:, :fw],
                        o_acc[i][:, f0:f0 + fw])

            # ================= schedule =================
            for q in range(NQ):
                for t in (0, 6):
                    emit_qk_unit(t, q, first=True)

            for h in range(H):
                c = h // 2
                fillers = []
                if h == 0:
                    fillers.append(("prelude",))
                if h < 10:  # QK chunk c+1: 2 units during each of h=2c, 2c+1
                    t0 = (c + 1, 6 + c + 1)
                    if h % 2 == 0:
                        fillers.append(("qk", t0[0], 0))
                        fillers.append(("qk", t0[1], 0))
                    else:
                        fillers.append(("qk", t0[0], 1))
                        fillers.append(("qk", t0[1], 1))
                if h in (1, 2):  # V projection (wv lands early now)
                    lo, hi = (0, 3) if h == 1 else (3, 8)
                    for i in range(lo, hi):
                        fillers.append(("v", i, 0))
                        fillers.append(("v", i, 1))
                if h >= 2:  # U for head h-2 (after V units at h=2)
                    for qc in range(NT):
                        fillers.append(("u", h - 2, qc))
                if h == 10:
                    fillers.append(("xfree",))
                if h in (6, 7):  # proj pair 0 (blocks 0,1; ready after U(3))
                    for i in range(4 * (h - 6), 4 * (h - 6) + 4):
                        fillers.append(("pa", i, 0, 0))
                        fillers.append(("pa", i, 1, 0))
                if h in (10, 11):  # proj pair 1 (blocks 2,3; after U(7))
                    for i in range(4 * (h - 10), 4 * (h - 10) + 4):
                        fillers.append(("pa", i, 0, 1))
                        fillers.append(("pa", i, 1, 1))
                if h == 11:  # U(10) late in head 11 (exp(10) done by then)
                    for qc in range(NT):
                        fillers.append(("u", 10, qc))

                fillers.sort(
                    key=lambda f: (f[0] == "u", f[0] == "prelude"))

                def drain(k):
                    for _ in range(k):
                        if not fillers:
                            return
                        f = fillers.pop(0)
                        if f[0] == "v":
                            emit_v_unit(f[1], f[2])
                        elif f[0] == "qk":
                            emit_qk_unit(f[1], f[2])
                        elif f[0] == "u":
                            emit_u_unit(f[1], f[2])
                            if f[1] == 3:
                                emit_transp(f[2], 0)
                            elif f[1] == 7:
                                emit_transp(f[2], 1)
                        elif f[0] == "pa":
                            emit_proj_unit(f[1], f[2], f[3])
                        elif f[0] == "xfree":
                            xp.release()
                        elif f[0] == "prelude":
                            emit_prelude()

                for kc in range(NT):
                    emit_s_unit(h, kc)
                    drain((len(fillers) + NT - kc - 1) // (NT - kc))
                drain(len(fillers))

            # ---- tail: U(11) + attnT blocks 4,5 + proj pair 2, pipelined
            def emit_out(i):
                po = ps.tile([128, 1024], F32, name=f"pot{i}", tag="s2",
                             bufs=2)
                for nf, (f0, fw) in enumerate(NFS):
                    for ci, cb in enumerate((4, 5)):
                        nc.tensor.matmul(
                            po[:, f0:f0 + fw],
                            attnTv[:, cb, i * 128:(i + 1) * 128],
                            wpb45v[:, cb - 4, f0:f0 + fw],
                            start=(ci == 0), stop=(ci == 1))
                nc.vector.tensor_add(
                    o_acc[i], po[:, 0:E], o_acc[i])
                nc.sync.dma_start(
                    out=out_d[i * 128:(i + 1) * 128, :], in_=o_acc[i])

            for qc in range(NT):
                emit_u_unit(11, qc)
                emit_transp(qc, 2)
            for qc in range(NT):
                emit_out(qc)
            iz.release()
            ep.release()
    nc.compile()
    return nc


_NC_CACHE = None


def kernel(x, W_qkv, b_qkv, W_proj, b_proj):
    from concourse.bass_utils import run_bass_kernel_spmd
    import ml_dtypes

    F8NP = ml_dtypes.float8_e4m3
    BF16NP = ml_dtypes.bfloat16

    global _NC_CACHE
    if _NC_CACHE is None:
        _NC_CACHE = _build()
    nc = _NC_CACHE

    x = np.asarray(x, dtype=np.float32)
    W_qkv = np.asarray(W_qkv, dtype=np.float32)
    b_qkv = np.ascontiguousarray(np.asarray(b_qkv, dtype=np.float32))
    W_proj = np.asarray(W_proj, dtype=np.float32)
    b_proj = np.asarray(b_proj, dtype=np.float32)

    # host-side preprocessing: transposed split-fp8 x, split-fp8 W_qkv,
    # bf16 W_proj and bias rows
    def split8(a):
        hi = a.astype(F8NP)
        lo = (a - hi.astype(np.float32)).astype(F8NP)
        return np.ascontiguousarray(hi), np.ascontiguousarray(lo)

    w_h, w_l = split8(W_qkv * np.float32(1024.0))
    wf_h = np.ascontiguousarray(
        np.concatenate([w_h[:, 0:128], w_h[:, 768:896]], axis=1))
    wf_l = np.ascontiguousarray(
        np.concatenate([w_l[:, 0:128], w_l[:, 768:896]], axis=1))
    xt32 = np.ascontiguousarray(np.swapaxes(x, 1, 2)) * np.float32(32.0)
    xsplit = [split8(xt32[b]) for b in range(B)]
    wp_h, wp_l = split8(W_proj * np.float32(1024.0))
    wp45 = np.ascontiguousarray(
        (W_proj[512:, :] * np.float32(1.0 / 32.0)).astype(BF16NP))
    bv_b = np.ascontiguousarray(b_qkv[2 * E:].astype(BF16NP))
    bp_b = np.ascontiguousarray(b_proj.astype(BF16NP))

    in_maps = [
        {"xh": xsplit[b][0], "xl": xsplit[b][1],
         "W_qkvh": w_h, "W_qkvl": w_l, "W_fh": wf_h, "W_fl": wf_l,
         "b_qkv": b_qkv,
         "b_v": bv_b, "W_projh": wp_h, "W_projl": wp_l, "W_pb45": wp45,
         "b_pb": bp_b}
        for b in range(B)
    ]
    res = run_bass_kernel_spmd(nc, in_maps, core_ids=list(range(B)))
    return np.stack([np.asarray(res.results[b]["out"]) for b in range(B)])


# revision 55
# speedup vs baseline: 1.0267x; 1.0180x over previous
"""Multi-head attention block on 8 Trainium2 NeuronCores.

Problem: B=8, N=1024, E=768, H=12, D=64 attention (QKV proj -> softmax(QK^T/8)V
-> output proj), fp32 I/O. Data parallel over batch: core b owns batch b.

v4 design (split-fp8 DoubleRow QKV + all-bf16 attention, host preprocessing):
  - Host precomputes transposed split-fp8 x (xh+xl ~= 32*x^T) and split-fp8
    W_qkv (wh+wl ~= 1024*W_qkv), bf16 W_proj / bias rows. All device loads are
    plain HWDGE DMAs (no casts, no SWDGE descriptor generation, no PE
    transposes of x).
  - QKV projection: 9 fp8 DoubleRow matmuls per psum tile ((xh+xl)@(wh+wl)
    with the xl@wl term dropped), 256-deep contraction pairs at 0.5 cyc/row.
    Psum carries 2^15 scale; Q/K evac rescales (tensor_scalar mult+add bias),
    V keeps the scale which cancels against the 2^15 ones-column in Z.
  - S^T[k,q] per head: two 512-wide bf16 matmuls into a [128,1024] psum
    (contraction d=64 at partition base (h%2)*64); exp on Act -> bf16 expS.
  - U restructured: stationary = expS chunk [128k,128q], moving = V [128k,65]
    (64 dims + 2^15 ones column) -> psum U^T[q,65] accumulated over k chunks;
    invZ = reciprocal of column 64 is a per-partition scalar; attn = U*invZ
    is one DVE tensor_scalar op. Halves U's PE rows vs the classic layout
    and kills the PE invZ broadcast.
  - attn rows (token-major) -> attnT (feature-major) via HWDGE XBAR DMA
    transposes (3 [128,128] bf16 blocks per DMA, zero PE cost).
  - Output proj split: attnT blocks 0..2 projected during late attention as
    PE filler; blocks 3..5 in the tail, accumulated into the same SBUF tile.
  - Emission interleaves S psum fills with QK/V/U/proj filler units so the
    Act engine (exp is ~100us of work, the secondary wall) starves as little
    as possible while PE (the primary wall) stays busy.
"""
import numpy as np

B, N, E, H, D = 8, 1024, 768, 12, 64
SCALE = D ** -0.5
NT = N // 128   # token chunks (8)
NE = E // 128   # embed chunks (6)
NQ = N // 512   # moving-dim tiles (2)
NFS = [(0, 512), (512, 256)]  # free-dim split of E for matmuls
PROJ_SPLIT = 3  # attnT blocks 0..2 in projA (during attention), 3..5 in tail


def _build():
    import concourse.bacc as bacc
    import concourse.mybir as mybir
    import concourse.tile as tile

    F32 = mybir.dt.float32
    BF16 = mybir.dt.bfloat16
    F8 = mybir.dt.float8e4
    EXP = mybir.ActivationFunctionType.Exp
    DR = mybir.MatmulPerfMode.DoubleRow
    MUL = mybir.AluOpType.mult
    ADD = mybir.AluOpType.add

    nc = bacc.Bacc("TRN2", target_bir_lowering=False)
    xh_d = nc.declare_dram_parameter("xh", [E, N], F8, isOutput=False)
    xl_d = nc.declare_dram_parameter("xl", [E, N], F8, isOutput=False)
    wqkvh_d = nc.declare_dram_parameter("W_qkvh", [E, 3 * E], F8, isOutput=False)
    wqkvl_d = nc.declare_dram_parameter("W_qkvl", [E, 3 * E], F8, isOutput=False)
    wfh_d = nc.declare_dram_parameter("W_fh", [E, 256], F8, isOutput=False)
    wfl_d = nc.declare_dram_parameter("W_fl", [E, 256], F8, isOutput=False)
    bqkv_d = nc.declare_dram_parameter("b_qkv", [3 * E], F32, isOutput=False)
    bv_d = nc.declare_dram_parameter("b_v", [E], BF16, isOutput=False)
    wprojh_d = nc.declare_dram_parameter("W_projh", [E, E], F8, isOutput=False)
    wprojl_d = nc.declare_dram_parameter("W_projl", [E, E], F8, isOutput=False)
    wpb45_d = nc.declare_dram_parameter("W_pb45", [256, E], BF16, isOutput=False)
    bp_d = nc.declare_dram_parameter("b_pb", [E], BF16, isOutput=False)
    out_d = nc.declare_dram_parameter("out", [N, E], F32, isOutput=True)

    with tile.TileContext(nc) as tc:
        with (
            tc.tile_pool(name="const", bufs=1) as cp,
            tc.tile_pool(name="main", bufs=1) as qp,
            tc.tile_pool(name="psum", bufs=1, space="PSUM") as ps,
        ):
            # ---- constants ----
            ones1 = cp.tile([1, 128], BF16)
            nc.vector.memset(ones1, 1.0)
            ones32k = cp.tile([1, 128], BF16)
            nc.vector.memset(ones32k, 32768.0)
            bqc = cp.tile([128, 12], F32)   # column fc = b_qkv[128fc:128(fc+1)]

            # ---- long-lived tensors ----
            qT = [qp.tile([128, N], BF16, name=f"qT{c}", tag=f"qT{c}")
                  for c in range(6)]
            kT = [qp.tile([128, N], BF16, name=f"kT{c}", tag=f"kT{c}")
                  for c in range(6)]
            vS = [qp.tile([128, 65 * H], BF16, name=f"vS{i}", tag=f"vS{i}")
                  for i in range(NT)]
            attnS = [qp.tile([128, E], BF16, name=f"atS{i}", tag=f"atS{i}")
                     for i in range(NT)]
            attnT = qp.tile([128, NE * N], BF16)  # [128, (c, 1024)] = 32*attn^T
            attnTv = attnT.rearrange("p (c n) -> p c n", n=N)
            attnTh = qp.tile([128, NE * N], F8)
            attnThv = attnTh.rearrange("p (c n) -> p c n", n=N)
            attnTl = qp.tile([128, NE * N], F8)
            attnTlv = attnTl.rearrange("p (c n) -> p c n", n=N)
            # W_proj pair tiles: pair pp = rows 256pp..256pp+256 as [128, 2, E]
            wphB = qp.tile([128, 2 * 2 * E], F8)
            wplB = qp.tile([128, 2 * 2 * E], F8)
            wphv = wphB.rearrange("p (pp t f) -> p pp t f", t=2, f=E)
            wplv = wplB.rearrange("p (pp t f) -> p pp t f", t=2, f=E)
            wpb45 = qp.tile([128, 2 * E], BF16)
            wpb45v = wpb45.rearrange("p (c f) -> p c f", f=E)
            bv_bc = qp.tile([128, E], F32)
            bp_bc = qp.tile([128, E], F32)
            bv_row = qp.tile([1, E], BF16)
            bp_row = qp.tile([1, E], BF16)
            o_acc = [qp.tile([128, E], F32, name=f"oa{i}", tag=f"oa{i}")
                     for i in range(NT)]

            # expS pool: [128, N] bf16 tiles; 3 heads alive (lag 2)
            ep = tc.alloc_tile_pool(name="exp", bufs=1)
            iz = tc.alloc_tile_pool(name="iz", bufs=1)

            # scoped pool: x / W_qkv fp8 tiles, released once QKV is done
            xp = tc.alloc_tile_pool(name="xw", bufs=1)
            xH = xp.tile([128, NE * N], F8)   # [128, (j, 1024 tok)] = 32*x^T
            xL = xp.tile([128, NE * N], F8)
            xHv = xH.rearrange("p (j n) -> p j n", n=N)
            xLv = xL.rearrange("p (j n) -> p j n", n=N)
            # weight pair big tiles: [128, (p, t, f)] with pair p = W rows
            # 256p..256p+256 split as 2 k-subtiles t
            wqkhB = xp.tile([128, 3 * 2 * 1536], F8)
            wqklB = xp.tile([128, 3 * 2 * 1536], F8)
            wqkhv = wqkhB.rearrange("p (pp t f) -> p pp t f", t=2, f=1536)
            wqklv = wqklB.rearrange("p (pp t f) -> p pp t f", t=2, f=1536)
            wqk0h = xp.tile([128, 3 * 2 * 256], F8)
            wqk0l = xp.tile([128, 3 * 2 * 256], F8)
            wqk0hv = wqk0h.rearrange("p (pp t f) -> p pp t f", t=2, f=256)
            wqk0lv = wqk0l.rearrange("p (pp t f) -> p pp t f", t=2, f=256)
            wvhB = xp.tile([128, 3 * 2 * E], F8)
            wvlB = xp.tile([128, 3 * 2 * E], F8)
            wvhv = wvhB.rearrange("p (pp t f) -> p pp t f", t=2, f=E)
            wvlv = wvlB.rearrange("p (pp t f) -> p pp t f", t=2, f=E)

            # ---- DMAs (all HWDGE, no casts): few big transfers, with two
            # small priority slices so the first S unit starts early ----
            def wqk_slice(wview, w_d, c0, cw):
                nc.sync.dma_start(
                    out=wview[:, :, :, c0:c0 + cw],
                    in_=w_d[0:768, c0:c0 + cw].rearrange(
                        "(pp t k) f -> k pp t f", t=2, k=128))

            # 1. host-packed t=0|t=6 weight columns (gate the first S unit)
            for w_t, w_d in ((wqk0h, wfh_d), (wqk0l, wfl_d)):
                nc.sync.dma_start(
                    out=w_t.rearrange("p (pp t f) -> p pp t f", t=2, f=256),
                    in_=w_d[:, :].rearrange(
                        "(pp t k) f -> k pp t f", t=2, k=128))
            # 2. x token-half 0, then half 1
            for half in range(2):
                t0 = half * 512
                nc.sync.dma_start(
                    out=xHv[:, :, t0:t0 + 512],
                    in_=xh_d[:, t0:t0 + 512].rearrange(
                        "(j k) n -> k j n", k=128))
                nc.sync.dma_start(
                    out=xLv[:, :, t0:t0 + 512],
                    in_=xl_d[:, t0:t0 + 512].rearrange(
                        "(j k) n -> k j n", k=128))
            nc.sync.dma_start(
                out=bqc, in_=bqkv_d[0:1536].rearrange("(f p) -> p f", p=128))
            nc.sync.dma_start(
                out=bv_row, in_=bv_d[:].rearrange("(o f) -> o f", o=1))
            nc.sync.dma_start(
                out=bp_row, in_=bp_d[:].rearrange("(o f) -> o f", o=1))
            # 3. remaining wqk columns
            wqk_slice(wqkhv, wqkvh_d, 128, 640)
            wqk_slice(wqklv, wqkvl_d, 128, 640)
            wqk_slice(wqkhv, wqkvh_d, 896, 640)
            wqk_slice(wqklv, wqkvl_d, 896, 640)
            # 4. V weights, bias rows, proj weights
            for w_t, w_d in ((wvhB, wqkvh_d), (wvlB, wqkvl_d)):
                nc.sync.dma_start(
                    out=w_t.rearrange("p (pp t f) -> p pp t f", t=2, f=E),
                    in_=w_d[0:768, 1536:].rearrange(
                        "(pp t k) f -> k pp t f", t=2, k=128))
            for w_t, w_d in ((wphB, wprojh_d), (wplB, wprojl_d)):
                nc.sync.dma_start(
                    out=w_t.rearrange("p (pp t f) -> p pp t f", t=2, f=E),
                    in_=w_d[0:512, :].rearrange(
                        "(pp t k) f -> k pp t f", t=2, k=128))
            nc.sync.dma_start(
                out=wpb45.rearrange("p (c f) -> p c f", f=E),
                in_=wpb45_d[:].rearrange("(c k) f -> k c f", k=128))

            def emit_prelude():
                for nf, (f0, fw) in enumerate(NFS):
                    pbv = ps.tile([128, 512], F32, name=f"pbv{nf}", tag="mm",
                                  bufs=2)
                    nc.tensor.matmul(pbv[:, :fw], ones32k,
                                     bv_row[:, f0:f0 + fw],
                                     start=True, stop=True)
                    nc.vector.tensor_copy(bv_bc[:, f0:f0 + fw], pbv[:, :fw])
                    pbp = ps.tile([128, 512], F32, name=f"pbp{nf}", tag="mm",
                                  bufs=2)
                    nc.tensor.matmul(pbp[:, :fw], ones32k,
                                     bp_row[:, f0:f0 + fw],
                                     start=True, stop=True)
                    nc.vector.tensor_copy(bp_bc[:, f0:f0 + fw], pbp[:, :fw])
                for i in range(NT):
                    nc.vector.memset(
                        vS[i].rearrange("p (h c) -> p h c", c=65)[:, :, 64:65],
                        1024.0)

            # ================= emission units =================
            def emit_qk_unit(t, q, evac_act=False, first=False):
                """One (feature-tile, 512-token-half) of Q or K projection.
                Split-fp8 DoubleRow: (xh+xl)@(wh+wl), xl@wl dropped."""
                dst = qT[t] if t < 6 else kT[t - 6]
                pq = ps.tile([128, 512], F32, name=f"pq{t}_{q}", tag="mm",
                             bufs=2)
                if first:
                    wcol0 = 0 if t == 0 else 128
                    terms = [(wqk0hv, xHv), (wqk0hv, xLv), (wqk0lv, xHv)]
                else:
                    wcol0 = t * 128
                    terms = [(wqkhv, xHv), (wqkhv, xLv), (wqklv, xHv)]
                nmm = 3 * len(terms)
                mi = 0
                for p in range(3):
                    for w_v, x_v in terms:
                        nc.tensor.matmul(
                            pq,
                            w_v[:, p, :, wcol0:wcol0 + 128],
                            x_v[:, 2 * p:2 * p + 2, q * 512:(q + 1) * 512],
                            start=(mi == 0), stop=(mi == nmm - 1),
                            perf_mode=DR)
                        mi += 1
                if evac_act:
                    nc.scalar.activation(
                        dst[:, q * 512:(q + 1) * 512], pq,
                        mybir.ActivationFunctionType.Identity,
                        bias=bqc[:, t:t + 1], scale=float(2.0 ** -15))
                else:
                    nc.vector.tensor_scalar(
                        out=dst[:, q * 512:(q + 1) * 512], in0=pq,
                        scalar1=float(2.0 ** -15), scalar2=bqc[:, t:t + 1],
                        op0=MUL, op1=ADD)

            def emit_v_unit(i, nf):
                """One (token-chunk, free-half) of the V projection."""
                f0, fw = NFS[nf]
                pv = ps.tile([128, 512], F32, name=f"pv{i}_{nf}", tag="mm",
                             bufs=2)
                terms = [(xHv, wvhv), (xHv, wvlv), (xLv, wvhv)]
                nmm = 3 * len(terms)
                mi = 0
                for p in range(3):
                    for x_v, w_v in terms:
                        nc.tensor.matmul(
                            pv[:, :fw],
                            x_v[:, 2 * p:2 * p + 2, i * 128:(i + 1) * 128],
                            w_v[:, p, :, f0:f0 + fw],
                            start=(mi == 0), stop=(mi == nmm - 1),
                            perf_mode=DR)
                        mi += 1
                nh, h0 = fw // D, f0 // D
                nc.vector.tensor_add(
                    vS[i].rearrange("p (h c) -> p h c", c=65)
                        [:, h0:h0 + nh, 0:64],
                    pv[:, :fw].rearrange("p (h d) -> p h d", d=D),
                    bv_bc[:, f0:f0 + fw].rearrange("p (h d) -> p h d", d=D))

            expS_of = {}

            def emit_s_unit(h, kc):
                """S^T[k-chunk, all q] for one head + exp -> bf16 expS."""
                c, r0 = h // 2, (h % 2) * 64
                if kc == 0:
                    expS_of[h] = [
                        ep.tile([128, N], BF16, name=f"eS{h}_{k2}",
                                tag="expS", bufs=24)
                        for k2 in range(NT)]
                pss = ps.tile([128, N], F32, name=f"ps{h}_{kc}", tag="s2",
                              bufs=2)
                for q in range(NQ):
                    nc.tensor.matmul(
                        pss[:, q * 512:(q + 1) * 512],
                        kT[c][r0:r0 + 64, kc * 128:(kc + 1) * 128],
                        qT[c][r0:r0 + 64, q * 512:(q + 1) * 512],
                        start=True, stop=True)
                    if h == 0 and kc == 0:
                        nc.scalar.activation(
                            expS_of[h][kc][:, q * 512:(q + 1) * 512],
                            pss[:, q * 512:(q + 1) * 512], EXP,
                            scale=float(SCALE))
                if not (h == 0 and kc == 0):
                    nc.scalar.activation(expS_of[h][kc], pss, EXP,
                                         scale=float(SCALE))

            def emit_u_unit(h, qc, evac_act=False):
                """U^T[q-chunk, 65] for one head; normalize into attnS."""
                expS = expS_of[h]
                pu = ps.tile([128, 512], F32, name=f"pu{h}_{qc}", tag="u",
                             bufs=2)
                for kc in range(NT):
                    nc.tensor.matmul(
                        pu[:, 0:65],
                        expS[kc][:, qc * 128:(qc + 1) * 128],
                        vS[kc][:, h * 65:h * 65 + 65],
                        start=(kc == 0), stop=(kc == NT - 1))
                invz = iz.tile([128, 1], F32, name=f"iv{h}_{qc}", tag="iz",
                               bufs=3)
                nc.vector.reciprocal(invz, pu[:, 64:65])
                if evac_act:
                    nc.scalar.activation(
                        attnS[qc][:, h * D:(h + 1) * D], pu[:, 0:64],
                        mybir.ActivationFunctionType.Copy, scale=invz)
                else:
                    nc.vector.tensor_scalar_mul(
                        attnS[qc][:, h * D:(h + 1) * D], pu[:, 0:64], invz)

            def emit_transp(qc, pp):
                """DMA-transpose one block pair into attnT (32*attn^T bf16),
                then split to fp8 hi/lo on the Pool engine."""
                c0 = 2 * pp
                t_bf = attnTv[:, c0:c0 + 2, qc * 128:(qc + 1) * 128]
                nc.sync.dma_start_transpose(
                    t_bf, attnS[qc][:, c0 * 128:(c0 + 2) * 128])
                if pp < 2:
                    t_hi = attnThv[:, c0:c0 + 2, qc * 128:(qc + 1) * 128]
                    nc.gpsimd.tensor_copy(t_hi, t_bf)
                    nc.gpsimd.tensor_sub(
                        attnTlv[:, c0:c0 + 2, qc * 128:(qc + 1) * 128],
                        t_bf, t_hi)

            def emit_proj_unit(i, nf, pp):
                """Half-row of output proj for block pair pp (split-fp8 DR).
                pp=0 adds the (2^15-scaled) bias; pp=1 accumulates; pp=2
                accumulates and rescales to the final f32 output."""
                f0, fw = NFS[nf]
                po = ps.tile([128, 512], F32, name=f"po{i}_{nf}_{pp}",
                             tag="mm", bufs=2)
                if pp < 2:
                    terms = [(attnThv, wphv), (attnThv, wplv), (attnTlv, wphv)]
                    for mi, (a_v, w_v) in enumerate(terms):
                        nc.tensor.matmul(
                            po[:, :fw],
                            a_v[:, 2 * pp:2 * pp + 2, i * 128:(i + 1) * 128],
                            w_v[:, pp, :, f0:f0 + fw],
                            start=(mi == 0), stop=(mi == len(terms) - 1),
                            perf_mode=DR)
                else:
                    for ci, cb in enumerate((4, 5)):
                        nc.tensor.matmul(
                            po[:, :fw],
                            attnTv[:, cb, i * 128:(i + 1) * 128],
                            wpb45v[:, cb - 4, f0:f0 + fw],
                            start=(ci == 0), stop=(ci == 1))
                if pp == 0:
                    nc.vector.tensor_add(
                        o_acc[i][:, f0:f0 + fw], po[:, :fw],
                        bp_bc[:, f0:f0 + fw])
                elif pp == 1:
                    nc.vector.tensor_add(
                        o_acc[i][:, f0:f0 + fw], po[:, :fw],
                        o_acc[i][:, f0:f0 + fw])
                    # rescale the pairs-0/1(+bias) partial here, off the tail
                    nc.gpsimd.tensor_scalar_mul(
                        o_acc[i][:, f0:f0 + fw], o_acc[i][:, f0:f0 + fw],
                        float(2.0 ** -15))
                else:
                    nc.vector.tensor_add(
                        o_acc[i][:, f0:f0 + fw], po[:, :fw],
                        o_acc[i][:, f0:f0 + fw])

            # ================= schedule =================
            for q in range(NQ):
                for t in (0, 6):
                    emit_qk_unit(t, q, first=True)

            for h in range(H):
                c = h // 2
                fillers = []
                if h == 0:
                    fillers.append(("prelude",))
                if h < 10:  # QK chunk c+1: 2 units during each of h=2c, 2c+1
                    t0 = (c + 1, 6 + c + 1)
                    if h % 2 == 0:
                        fillers.append(("qk", t0[0], 0))
                        fillers.append(("qk", t0[1], 0))
                    else:
                        fillers.append(("qk", t0[0], 1))
                        fillers.append(("qk", t0[1], 1))
                # V projection: nf0 (heads 0-7 columns, needed by U(0) at
                # h2) early; nf1 (heads 8-11, first needed by U(8) at h10)
                # in the slack heads 4-5.
                if h in (1, 2):
                    lo, hi = (0, 4) if h == 1 else (4, 8)
                    for i in range(lo, hi):
                        fillers.append(("v", i, 0))
                if h in (4, 5, 6):
                    lo, hi = ((0, 3), (3, 6), (6, 8))[h - 4]
                    for i in range(lo, hi):
                        fillers.append(("v", i, 1))
                if h >= 2:  # U for head h-2 (after V units at h=2)
                    for qc in range(NT):
                        fillers.append(("u", h - 2, qc))
                if h == 10:
                    fillers.append(("xfree",))
                if h in (6, 7, 8):  # proj pair 0 (ready after U(3))
                    lo, hi = ((0, 3), (3, 6), (6, 8))[h - 6]
                    for i in range(lo, hi):
                        fillers.append(("pa", i, 0, 0))
                        fillers.append(("pa", i, 1, 0))
                if h in (10, 11):  # proj pair 1 (blocks 2,3; after U(7))
                    lo, hi = (0, 4) if h == 10 else (4, 8)
                    for i in range(lo, hi):
                        fillers.append(("pa", i, 0, 1))
                        fillers.append(("pa", i, 1, 1))
                if h == 11:  # U(10) late in head 11 (exp(10) done by then)
                    for qc in range(NT):
                        fillers.append(("u", 10, qc))

                fillers.sort(
                    key=lambda f: (f[0] == "u", f[0] == "prelude"))

                def drain(k):
                    for _ in range(k):
                        if not fillers:
                            return
                        f = fillers.pop(0)
                        if f[0] == "v":
                            emit_v_unit(f[1], f[2])
                        elif f[0] == "qk":
                            emit_qk_unit(f[1], f[2])
                        elif f[0] == "u":
                            emit_u_unit(f[1], f[2])
                            if f[1] == 3:
                                emit_transp(f[2], 0)
                            elif f[1] == 7:
                                emit_transp(f[2], 1)
                        elif f[0] == "pa":
                            emit_proj_unit(f[1], f[2], f[3])
                        elif f[0] == "xfree":
                            xp.release()
                        elif f[0] == "prelude":
                            emit_prelude()

                for kc in range(NT):
                    emit_s_unit(h, kc)
                    drain((len(fillers) + NT - kc - 1) // (NT - kc))
                drain(len(fillers))

            # ---- tail: U(11) + attnT blocks 4,5 + proj pair 2, pipelined
            def emit_out(i):
                po = ps.tile([128, 1024], F32, name=f"pot{i}", tag="s2",
                             bufs=2)
                for nf, (f0, fw) in enumerate(NFS):
                    for ci, cb in enumerate((4, 5)):
                        nc.tensor.matmul(
                            po[:, f0:f0 + fw],
                            attnTv[:, cb, i * 128:(i + 1) * 128],
                            wpb45v[:, cb - 4, f0:f0 + fw],
                            start=(ci == 0), stop=(ci == 1))
                nc.vector.tensor_add(
                    o_acc[i], po[:, 0:E], o_acc[i])
                nc.sync.dma_start(
                    out=out_d[i * 128:(i + 1) * 128, :], in_=o_acc[i])

            for qc in range(NT):
                emit_u_unit(11, qc)
                emit_transp(qc, 2)
            for qc in range(NT):
                emit_out(qc)
            iz.release()
            ep.release()
    nc.compile()
    return nc


_NC_CACHE = None


def kernel(x, W_qkv, b_qkv, W_proj, b_proj):
    from concourse.bass_utils import run_bass_kernel_spmd
    import ml_dtypes

    F8NP = ml_dtypes.float8_e4m3
    BF16NP = ml_dtypes.bfloat16

    global _NC_CACHE
    if _NC_CACHE is None:
        _NC_CACHE = _build()
    nc = _NC_CACHE

    x = np.asarray(x, dtype=np.float32)
    W_qkv = np.asarray(W_qkv, dtype=np.float32)
    b_qkv = np.ascontiguousarray(np.asarray(b_qkv, dtype=np.float32))
    W_proj = np.asarray(W_proj, dtype=np.float32)
    b_proj = np.asarray(b_proj, dtype=np.float32)

    # host-side preprocessing: transposed split-fp8 x, split-fp8 W_qkv,
    # bf16 W_proj and bias rows
    def split8(a):
        hi = a.astype(F8NP)
        lo = (a - hi.astype(np.float32)).astype(F8NP)
        return np.ascontiguousarray(hi), np.ascontiguousarray(lo)

    w_h, w_l = split8(W_qkv * np.float32(1024.0))
    wf_h = np.ascontiguousarray(
        np.concatenate([w_h[:, 0:128], w_h[:, 768:896]], axis=1))
    wf_l = np.ascontiguousarray(
        np.concatenate([w_l[:, 0:128], w_l[:, 768:896]], axis=1))
    xt32 = np.ascontiguousarray(np.swapaxes(x, 1, 2)) * np.float32(32.0)
    xsplit = [split8(xt32[b]) for b in range(B)]
    wp_h, wp_l = split8(W_proj * np.float32(1024.0))
    wp45 = np.ascontiguousarray(
        (W_proj[512:, :] * np.float32(1.0 / 32.0)).astype(BF16NP))
    bv_b = np.ascontiguousarray(b_qkv[2 * E:].astype(BF16NP))
    bp_b = np.ascontiguousarray(b_proj.astype(BF16NP))

    in_maps = [
        {"xh": xsplit[b][0], "xl": xsplit[b][1],
         "W_qkvh": w_h, "W_qkvl": w_l, "W_fh": wf_h, "W_fl": wf_l,
         "b_qkv": b_qkv,
         "b_v": bv_b, "W_projh": wp_h, "W_projl": wp_l, "W_pb45": wp45,
         "b_pb": bp_b}
        for b in range(B)
    ]
    res = run_bass_kernel_spmd(nc, in_maps, core_ids=list(range(B)))
    return np.stack([np.asarray(res.results[b]["out"]) for b in range(B)])


# revision 56
# speedup vs baseline: 1.0268x; 1.0001x over previous
"""Multi-head attention block on 8 Trainium2 NeuronCores.

Problem: B=8, N=1024, E=768, H=12, D=64 attention (QKV proj -> softmax(QK^T/8)V
-> output proj), fp32 I/O. Data parallel over batch: core b owns batch b.

v4 design (split-fp8 DoubleRow QKV + all-bf16 attention, host preprocessing):
  - Host precomputes transposed split-fp8 x (xh+xl ~= 32*x^T) and split-fp8
    W_qkv (wh+wl ~= 1024*W_qkv), bf16 W_proj / bias rows. All device loads are
    plain HWDGE DMAs (no casts, no SWDGE descriptor generation, no PE
    transposes of x).
  - QKV projection: 9 fp8 DoubleRow matmuls per psum tile ((xh+xl)@(wh+wl)
    with the xl@wl term dropped), 256-deep contraction pairs at 0.5 cyc/row.
    Psum carries 2^15 scale; Q/K evac rescales (tensor_scalar mult+add bias),
    V keeps the scale which cancels against the 2^15 ones-column in Z.
  - S^T[k,q] per head: two 512-wide bf16 matmuls into a [128,1024] psum
    (contraction d=64 at partition base (h%2)*64); exp on Act -> bf16 expS.
  - U restructured: stationary = expS chunk [128k,128q], moving = V [128k,65]
    (64 dims + 2^15 ones column) -> psum U^T[q,65] accumulated over k chunks;
    invZ = reciprocal of column 64 is a per-partition scalar; attn = U*invZ
    is one DVE tensor_scalar op. Halves U's PE rows vs the classic layout
    and kills the PE invZ broadcast.
  - attn rows (token-major) -> attnT (feature-major) via HWDGE XBAR DMA
    transposes (3 [128,128] bf16 blocks per DMA, zero PE cost).
  - Output proj split: attnT blocks 0..2 projected during late attention as
    PE filler; blocks 3..5 in the tail, accumulated into the same SBUF tile.
  - Emission interleaves S psum fills with QK/V/U/proj filler units so the
    Act engine (exp is ~100us of work, the secondary wall) starves as little
    as possible while PE (the primary wall) stays busy.
"""
import numpy as np

B, N, E, H, D = 8, 1024, 768, 12, 64
SCALE = D ** -0.5
NT = N // 128   # token chunks (8)
NE = E // 128   # embed chunks (6)
NQ = N // 512   # moving-dim tiles (2)
NFS = [(0, 512), (512, 256)]  # free-dim split of E for matmuls
PROJ_SPLIT = 3  # attnT blocks 0..2 in projA (during attention), 3..5 in tail


def _build():
    import concourse.bacc as bacc
    import concourse.mybir as mybir
    import concourse.tile as tile

    F32 = mybir.dt.float32
    BF16 = mybir.dt.bfloat16
    F8 = mybir.dt.float8e4
    EXP = mybir.ActivationFunctionType.Exp
    DR = mybir.MatmulPerfMode.DoubleRow
    MUL = mybir.AluOpType.mult
    ADD = mybir.AluOpType.add

    nc = bacc.Bacc("TRN2", target_bir_lowering=False)
    xh_d = nc.declare_dram_parameter("xh", [E, N], F8, isOutput=False)
    xl_d = nc.declare_dram_parameter("xl", [E, N], F8, isOutput=False)
    wqkvh_d = nc.declare_dram_parameter("W_qkvh", [E, 3 * E], F8, isOutput=False)
    wqkvl_d = nc.declare_dram_parameter("W_qkvl", [E, 3 * E], F8, isOutput=False)
    wfh_d = nc.declare_dram_parameter("W_fh", [E, 256], F8, isOutput=False)
    wfl_d = nc.declare_dram_parameter("W_fl", [E, 256], F8, isOutput=False)
    bqkv_d = nc.declare_dram_parameter("b_qkv", [3 * E], F32, isOutput=False)
    bv_d = nc.declare_dram_parameter("b_v", [E], BF16, isOutput=False)
    wprojh_d = nc.declare_dram_parameter("W_projh", [E, E], F8, isOutput=False)
    wprojl_d = nc.declare_dram_parameter("W_projl", [E, E], F8, isOutput=False)
    wpb45_d = nc.declare_dram_parameter("W_pb45", [256, E], BF16, isOutput=False)
    bp_d = nc.declare_dram_parameter("b_pb", [E], BF16, isOutput=False)
    out_d = nc.declare_dram_parameter("out", [N, E], F32, isOutput=True)

    with tile.TileContext(nc) as tc:
        with (
            tc.tile_pool(name="const", bufs=1) as cp,
            tc.tile_pool(name="main", bufs=1) as qp,
            tc.tile_pool(name="psum", bufs=1, space="PSUM") as ps,
        ):
            # ---- constants ----
            ones1 = cp.tile([1, 128], BF16)
            nc.vector.memset(ones1, 1.0)
            ones32k = cp.tile([1, 128], BF16)
            nc.vector.memset(ones32k, 32768.0)
            bqc = cp.tile([128, 12], F32)   # column fc = b_qkv[128fc:128(fc+1)]

            # ---- long-lived tensors ----
            qT = [qp.tile([128, N], BF16, name=f"qT{c}", tag=f"qT{c}")
                  for c in range(6)]
            kT = [qp.tile([128, N], BF16, name=f"kT{c}", tag=f"kT{c}")
                  for c in range(6)]
            vS = [qp.tile([128, 65 * H], BF16, name=f"vS{i}", tag=f"vS{i}")
                  for i in range(NT)]
            attnS = [qp.tile([128, E], BF16, name=f"atS{i}", tag=f"atS{i}")
                     for i in range(NT)]
            attnT = qp.tile([128, NE * N], BF16)  # [128, (c, 1024)] = 32*attn^T
            attnTv = attnT.rearrange("p (c n) -> p c n", n=N)
            attnTh = qp.tile([128, NE * N], F8)
            attnThv = attnTh.rearrange("p (c n) -> p c n", n=N)
            attnTl = qp.tile([128, NE * N], F8)
            attnTlv = attnTl.rearrange("p (c n) -> p c n", n=N)
            # W_proj pair tiles: pair pp = rows 256pp..256pp+256 as [128, 2, E]
            wphB = qp.tile([128, 2 * 2 * E], F8)
            wplB = qp.tile([128, 2 * 2 * E], F8)
            wphv = wphB.rearrange("p (pp t f) -> p pp t f", t=2, f=E)
            wplv = wplB.rearrange("p (pp t f) -> p pp t f", t=2, f=E)
            wpb45 = qp.tile([128, 2 * E], BF16)
            wpb45v = wpb45.rearrange("p (c f) -> p c f", f=E)
            bv_bc = qp.tile([128, E], F32)
            bp_bc = qp.tile([128, E], F32)
            bv_row = qp.tile([1, E], BF16)
            bp_row = qp.tile([1, E], BF16)
            o_acc = [qp.tile([128, E], F32, name=f"oa{i}", tag=f"oa{i}")
                     for i in range(NT)]

            # expS pool: [128, N] bf16 tiles; 3 heads alive (lag 2)
            ep = tc.alloc_tile_pool(name="exp", bufs=1)
            iz = tc.alloc_tile_pool(name="iz", bufs=1)

            # scoped pool: x / W_qkv fp8 tiles, released once QKV is done
            xp = tc.alloc_tile_pool(name="xw", bufs=1)
            xH = xp.tile([128, NE * N], F8)   # [128, (j, 1024 tok)] = 32*x^T
            xL = xp.tile([128, NE * N], F8)
            xHv = xH.rearrange("p (j n) -> p j n", n=N)
            xLv = xL.rearrange("p (j n) -> p j n", n=N)
            # weight pair big tiles: [128, (p, t, f)] with pair p = W rows
            # 256p..256p+256 split as 2 k-subtiles t
            wqkhB = xp.tile([128, 3 * 2 * 1536], F8)
            wqklB = xp.tile([128, 3 * 2 * 1536], F8)
            wqkhv = wqkhB.rearrange("p (pp t f) -> p pp t f", t=2, f=1536)
            wqklv = wqklB.rearrange("p (pp t f) -> p pp t f", t=2, f=1536)
            wqk0h = xp.tile([128, 3 * 2 * 256], F8)
            wqk0l = xp.tile([128, 3 * 2 * 256], F8)
            wqk0hv = wqk0h.rearrange("p (pp t f) -> p pp t f", t=2, f=256)
            wqk0lv = wqk0l.rearrange("p (pp t f) -> p pp t f", t=2, f=256)
            wvhB = xp.tile([128, 3 * 2 * E], F8)
            wvlB = xp.tile([128, 3 * 2 * E], F8)
            wvhv = wvhB.rearrange("p (pp t f) -> p pp t f", t=2, f=E)
            wvlv = wvlB.rearrange("p (pp t f) -> p pp t f", t=2, f=E)

            # ---- DMAs (all HWDGE, no casts): few big transfers, with two
            # small priority slices so the first S unit starts early ----
            def wqk_slice(wview, w_d, c0, cw):
                nc.sync.dma_start(
                    out=wview[:, :, :, c0:c0 + cw],
                    in_=w_d[0:768, c0:c0 + cw].rearrange(
                        "(pp t k) f -> k pp t f", t=2, k=128))

            # 1. host-packed t=0|t=6 weight columns (gate the first S unit)
            for w_t, w_d in ((wqk0h, wfh_d), (wqk0l, wfl_d)):
                nc.sync.dma_start(
                    out=w_t.rearrange("p (pp t f) -> p pp t f", t=2, f=256),
                    in_=w_d[:, :].rearrange(
                        "(pp t k) f -> k pp t f", t=2, k=128))
            # 2. x token-half 0, then half 1
            for half in range(2):
                t0 = half * 512
                nc.sync.dma_start(
                    out=xHv[:, :, t0:t0 + 512],
                    in_=xh_d[:, t0:t0 + 512].rearrange(
                        "(j k) n -> k j n", k=128))
                nc.sync.dma_start(
                    out=xLv[:, :, t0:t0 + 512],
                    in_=xl_d[:, t0:t0 + 512].rearrange(
                        "(j k) n -> k j n", k=128))
            nc.sync.dma_start(
                out=bqc, in_=bqkv_d[0:1536].rearrange("(f p) -> p f", p=128))
            nc.sync.dma_start(
                out=bv_row, in_=bv_d[:].rearrange("(o f) -> o f", o=1))
            nc.sync.dma_start(
                out=bp_row, in_=bp_d[:].rearrange("(o f) -> o f", o=1))
            # 3. remaining wqk columns
            wqk_slice(wqkhv, wqkvh_d, 128, 640)
            wqk_slice(wqklv, wqkvl_d, 128, 640)
            wqk_slice(wqkhv, wqkvh_d, 896, 640)
            wqk_slice(wqklv, wqkvl_d, 896, 640)
            # 4. V weights, bias rows, proj weights
            for w_t, w_d in ((wvhB, wqkvh_d), (wvlB, wqkvl_d)):
                nc.sync.dma_start(
                    out=w_t.rearrange("p (pp t f) -> p pp t f", t=2, f=E),
                    in_=w_d[0:768, 1536:].rearrange(
                        "(pp t k) f -> k pp t f", t=2, k=128))
            for w_t, w_d in ((wphB, wprojh_d), (wplB, wprojl_d)):
                nc.sync.dma_start(
                    out=w_t.rearrange("p (pp t f) -> p pp t f", t=2, f=E),
                    in_=w_d[0:512, :].rearrange(
                        "(pp t k) f -> k pp t f", t=2, k=128))
            nc.sync.dma_start(
                out=wpb45.rearrange("p (c f) -> p c f", f=E),
                in_=wpb45_d[:].rearrange("(c k) f -> k c f", k=128))

            def emit_prelude():
                for nf, (f0, fw) in enumerate(NFS):
                    pbv = ps.tile([128, 512], F32, name=f"pbv{nf}", tag="mm",
                                  bufs=2)
                    nc.tensor.matmul(pbv[:, :fw], ones32k,
                                     bv_row[:, f0:f0 + fw],
                                     start=True, stop=True)
                    nc.vector.tensor_copy(bv_bc[:, f0:f0 + fw], pbv[:, :fw])
                    pbp = ps.tile([128, 512], F32, name=f"pbp{nf}", tag="mm",
                                  bufs=2)
                    nc.tensor.matmul(pbp[:, :fw], ones32k,
                                     bp_row[:, f0:f0 + fw],
                                     start=True, stop=True)
                    nc.vector.tensor_copy(bp_bc[:, f0:f0 + fw], pbp[:, :fw])
                for i in range(NT):
                    nc.vector.memset(
                        vS[i].rearrange("p (h c) -> p h c", c=65)[:, :, 64:65],
                        1024.0)

            # ================= emission units =================
            def emit_qk_unit(t, q, evac_act=False, first=False):
                """One (feature-tile, 512-token-half) of Q or K projection.
                Split-fp8 DoubleRow: (xh+xl)@(wh+wl), xl@wl dropped."""
                dst = qT[t] if t < 6 else kT[t - 6]
                pq = ps.tile([128, 512], F32, name=f"pq{t}_{q}", tag="mm",
                             bufs=2)
                if first:
                    wcol0 = 0 if t == 0 else 128
                    terms = [(wqk0hv, xHv), (wqk0hv, xLv), (wqk0lv, xHv)]
                else:
                    wcol0 = t * 128
                    terms = [(wqkhv, xHv), (wqkhv, xLv), (wqklv, xHv)]
                nmm = 3 * len(terms)
                mi = 0
                for p in range(3):
                    for w_v, x_v in terms:
                        nc.tensor.matmul(
                            pq,
                            w_v[:, p, :, wcol0:wcol0 + 128],
                            x_v[:, 2 * p:2 * p + 2, q * 512:(q + 1) * 512],
                            start=(mi == 0), stop=(mi == nmm - 1),
                            perf_mode=DR)
                        mi += 1
                if evac_act:
                    nc.scalar.activation(
                        dst[:, q * 512:(q + 1) * 512], pq,
                        mybir.ActivationFunctionType.Identity,
                        bias=bqc[:, t:t + 1], scale=float(2.0 ** -15))
                else:
                    nc.vector.tensor_scalar(
                        out=dst[:, q * 512:(q + 1) * 512], in0=pq,
                        scalar1=float(2.0 ** -15), scalar2=bqc[:, t:t + 1],
                        op0=MUL, op1=ADD)

            def emit_v_unit(i, nf):
                """One (token-chunk, free-half) of the V projection."""
                f0, fw = NFS[nf]
                pv = ps.tile([128, 512], F32, name=f"pv{i}_{nf}", tag="mm",
                             bufs=2)
                terms = [(xHv, wvhv), (xHv, wvlv), (xLv, wvhv)]
                nmm = 3 * len(terms)
                mi = 0
                for p in range(3):
                    for x_v, w_v in terms:
                        nc.tensor.matmul(
                            pv[:, :fw],
                            x_v[:, 2 * p:2 * p + 2, i * 128:(i + 1) * 128],
                            w_v[:, p, :, f0:f0 + fw],
                            start=(mi == 0), stop=(mi == nmm - 1),
                            perf_mode=DR)
                        mi += 1
                nh, h0 = fw // D, f0 // D
                nc.vector.tensor_add(
                    vS[i].rearrange("p (h c) -> p h c", c=65)
                        [:, h0:h0 + nh, 0:64],
                    pv[:, :fw].rearrange("p (h d) -> p h d", d=D),
                    bv_bc[:, f0:f0 + fw].rearrange("p (h d) -> p h d", d=D))

            expS_of = {}

            def emit_s_unit(h, kc):
                """S^T[k-chunk, all q] for one head + exp -> bf16 expS."""
                c, r0 = h // 2, (h % 2) * 64
                if kc == 0:
                    expS_of[h] = [
                        ep.tile([128, N], BF16, name=f"eS{h}_{k2}",
                                tag="expS", bufs=24)
                        for k2 in range(NT)]
                pss = ps.tile([128, N], F32, name=f"ps{h}_{kc}", tag="s2",
                              bufs=2)
                for q in range(NQ):
                    nc.tensor.matmul(
                        pss[:, q * 512:(q + 1) * 512],
                        kT[c][r0:r0 + 64, kc * 128:(kc + 1) * 128],
                        qT[c][r0:r0 + 64, q * 512:(q + 1) * 512],
                        start=True, stop=True)
                    if h == 0 and kc == 0:
                        nc.scalar.activation(
                            expS_of[h][kc][:, q * 512:(q + 1) * 512],
                            pss[:, q * 512:(q + 1) * 512], EXP,
                            scale=float(SCALE))
                if not (h == 0 and kc == 0):
                    nc.scalar.activation(expS_of[h][kc], pss, EXP,
                                         scale=float(SCALE))

            def emit_u_unit(h, qc, evac_act=False):
                """U^T[q-chunk, 65] for one head; normalize into attnS."""
                expS = expS_of[h]
                pu = ps.tile([128, 512], F32, name=f"pu{h}_{qc}", tag="u",
                             bufs=2)
                for kc in range(NT):
                    nc.tensor.matmul(
                        pu[:, 0:65],
                        expS[kc][:, qc * 128:(qc + 1) * 128],
                        vS[kc][:, h * 65:h * 65 + 65],
                        start=(kc == 0), stop=(kc == NT - 1))
                invz = iz.tile([128, 1], F32, name=f"iv{h}_{qc}", tag="iz",
                               bufs=3)
                nc.vector.reciprocal(invz, pu[:, 64:65])
                if evac_act:
                    nc.scalar.activation(
                        attnS[qc][:, h * D:(h + 1) * D], pu[:, 0:64],
                        mybir.ActivationFunctionType.Copy, scale=invz)
                else:
                    nc.vector.tensor_scalar_mul(
                        attnS[qc][:, h * D:(h + 1) * D], pu[:, 0:64], invz)

            def emit_transp(qc, pp):
                """DMA-transpose one block pair into attnT (32*attn^T bf16),
                then split to fp8 hi/lo on the Pool engine."""
                c0 = 2 * pp
                t_bf = attnTv[:, c0:c0 + 2, qc * 128:(qc + 1) * 128]
                nc.sync.dma_start_transpose(
                    t_bf, attnS[qc][:, c0 * 128:(c0 + 2) * 128])
                if pp < 2:
                    t_hi = attnThv[:, c0:c0 + 2, qc * 128:(qc + 1) * 128]
                    nc.gpsimd.tensor_copy(t_hi, t_bf)
                    nc.gpsimd.tensor_sub(
                        attnTlv[:, c0:c0 + 2, qc * 128:(qc + 1) * 128],
                        t_bf, t_hi)

            def emit_proj_unit(i, nf, pp):
                """Half-row of output proj for block pair pp (split-fp8 DR).
                pp=0 adds the (2^15-scaled) bias; pp=1 accumulates; pp=2
                accumulates and rescales to the final f32 output."""
                f0, fw = NFS[nf]
                po = ps.tile([128, 512], F32, name=f"po{i}_{nf}_{pp}",
                             tag="mm", bufs=2)
                if pp < 2:
                    terms = [(attnThv, wphv), (attnThv, wplv), (attnTlv, wphv)]
                    for mi, (a_v, w_v) in enumerate(terms):
                        nc.tensor.matmul(
                            po[:, :fw],
                            a_v[:, 2 * pp:2 * pp + 2, i * 128:(i + 1) * 128],
                            w_v[:, pp, :, f0:f0 + fw],
                            start=(mi == 0), stop=(mi == len(terms) - 1),
                            perf_mode=DR)
                else:
                    for ci, cb in enumerate((4, 5)):
                        nc.tensor.matmul(
                            po[:, :fw],
                            attnTv[:, cb, i * 128:(i + 1) * 128],
                            wpb45v[:, cb - 4, f0:f0 + fw],
                            start=(ci == 0), stop=(ci == 1))
                if pp == 0:
                    nc.vector.tensor_add(
                        o_acc[i][:, f0:f0 + fw], po[:, :fw],
                        bp_bc[:, f0:f0 + fw])
                elif pp == 1:
                    nc.vector.tensor_add(
                        o_acc[i][:, f0:f0 + fw], po[:, :fw],
                        o_acc[i][:, f0:f0 + fw])
                    # rescale the pairs-0/1(+bias) partial here, off the tail
                    nc.gpsimd.tensor_scalar_mul(
                        o_acc[i][:, f0:f0 + fw], o_acc[i][:, f0:f0 + fw],
                        float(2.0 ** -15))
                else:
                    nc.vector.tensor_add(
                        o_acc[i][:, f0:f0 + fw], po[:, :fw],
                        o_acc[i][:, f0:f0 + fw])

            # ================= schedule =================
            for q in range(NQ):
                for t in (0, 6):
                    emit_qk_unit(t, q, first=True)

            for h in range(H):
                c = h // 2
                fillers = []
                if h == 0:
                    fillers.append(("prelude",))
                if h < 10:  # QK chunk c+1: 2 units during each of h=2c, 2c+1
                    t0 = (c + 1, 6 + c + 1)
                    if h % 2 == 0:
                        fillers.append(("qk", t0[0], 0))
                        fillers.append(("qk", t0[1], 0))
                    else:
                        fillers.append(("qk", t0[0], 1))
                        fillers.append(("qk", t0[1], 1))
                # V projection: nf0 (heads 0-7 columns, needed by U(0) at
                # h2) early; nf1 (heads 8-11, first needed by U(8) at h10)
                # in the slack heads 4-5.
                if h in (1, 2):
                    lo, hi = (0, 4) if h == 1 else (4, 8)
                    for i in range(lo, hi):
                        fillers.append(("v", i, 0))
                if h in (4, 5, 6):
                    lo, hi = ((0, 3), (3, 6), (6, 8))[h - 4]
                    for i in range(lo, hi):
                        fillers.append(("v", i, 1))
                if h >= 2:  # U for head h-2 (after V units at h=2)
                    for qc in range(NT):
                        fillers.append(("u", h - 2, qc))
                if h == 10:
                    fillers.append(("xfree",))
                if h in (6, 7, 8):  # proj pair 0 (ready after U(3))
                    lo, hi = ((0, 3), (3, 6), (6, 8))[h - 6]
                    for i in range(lo, hi):
                        fillers.append(("pa", i, 0, 0))
                        fillers.append(("pa", i, 1, 0))
                if h in (10, 11):  # proj pair 1 (blocks 2,3; after U(7))
                    lo, hi = (0, 4) if h == 10 else (4, 8)
                    for i in range(lo, hi):
                        fillers.append(("pa", i, 0, 1))
                        fillers.append(("pa", i, 1, 1))
                if h == 11:  # U(10) late in head 11 (exp(10) done by then)
                    for qc in range(NT):
                        fillers.append(("u", 10, qc))

                fillers.sort(
                    key=lambda f: (f[0] == "u", f[0] != "prelude"))

                def drain(k):
                    for _ in range(k):
                        if not fillers:
                            return
                        f = fillers.pop(0)
                        if f[0] == "v":
                            emit_v_unit(f[1], f[2])
                        elif f[0] == "qk":
                            emit_qk_unit(f[1], f[2])
                        elif f[0] == "u":
                            emit_u_unit(f[1], f[2])
                            if f[1] == 3:
                                emit_transp(f[2], 0)
                            elif f[1] == 7:
                                emit_transp(f[2], 1)
                        elif f[0] == "pa":
                            emit_proj_unit(f[1], f[2], f[3])
                        elif f[0] == "xfree":
                            xp.release()
                        elif f[0] == "prelude":
                            emit_prelude()

                for kc in range(NT):
                    emit_s_unit(h, kc)
                    drain((len(fillers) + NT - kc - 1) // (NT - kc))
                drain(len(fillers))

            # ---- tail: U(11) + attnT blocks 4,5 + proj pair 2, pipelined
            def emit_out(i):
                po = ps.tile([128, 1024], F32, name=f"pot{i}", tag="s2",
                             bufs=2)
                for nf, (f0, fw) in enumerate(NFS):
                    for ci, cb in enumerate((4, 5)):
                        nc.tensor.matmul(
                            po[:, f0:f0 + fw],
                            attnTv[:, cb, i * 128:(i + 1) * 128],
                            wpb45v[:, cb - 4, f0:f0 + fw],
                            start=(ci == 0), stop=(ci == 1))
                nc.vector.tensor_add(
                    o_acc[i], po[:, 0:E], o_acc[i])
                nc.sync.dma_start(
                    out=out_d[i * 128:(i + 1) * 128, :], in_=o_acc[i])

            for qc in range(NT):
                emit_u_unit(11, qc)
                emit_transp(qc, 2)
            for qc in range(NT):
                emit_out(qc)
            iz.release()
            ep.release()
    nc.compile()
    return nc


_NC_CACHE = None


def kernel(x, W_qkv, b_qkv, W_proj, b_proj):
    from concourse.bass_utils import run_bass_kernel_spmd
    import ml_dtypes

    F8NP = ml_dtypes.float8_e4m3
    BF16NP = ml_dtypes.bfloat16

    global _NC_CACHE
    if _NC_CACHE is None:
        _NC_CACHE = _build()
    nc = _NC_CACHE

    x = np.asarray(x, dtype=np.float32)
    W_qkv = np.asarray(W_qkv, dtype=np.float32)
    b_qkv = np.ascontiguousarray(np.asarray(b_qkv, dtype=np.float32))
    W_proj = np.asarray(W_proj, dtype=np.float32)
    b_proj = np.asarray(b_proj, dtype=np.float32)

    # host-side preprocessing: transposed split-fp8 x, split-fp8 W_qkv,
    # bf16 W_proj and bias rows
    def split8(a):
        hi = a.astype(F8NP)
        lo = (a - hi.astype(np.float32)).astype(F8NP)
        return np.ascontiguousarray(hi), np.ascontiguousarray(lo)

    w_h, w_l = split8(W_qkv * np.float32(1024.0))
    wf_h = np.ascontiguousarray(
        np.concatenate([w_h[:, 0:128], w_h[:, 768:896]], axis=1))
    wf_l = np.ascontiguousarray(
        np.concatenate([w_l[:, 0:128], w_l[:, 768:896]], axis=1))
    xt32 = np.ascontiguousarray(np.swapaxes(x, 1, 2)) * np.float32(32.0)
    xsplit = [split8(xt32[b]) for b in range(B)]
    wp_h, wp_l = split8(W_proj * np.float32(1024.0))
    wp45 = np.ascontiguousarray(
        (W_proj[512:, :] * np.float32(1.0 / 32.0)).astype(BF16NP))
    bv_b = np.ascontiguousarray(b_qkv[2 * E:].astype(BF16NP))
    bp_b = np.ascontiguousarray(b_proj.astype(BF16NP))

    in_maps = [
        {"xh": xsplit[b][0], "xl": xsplit[b][1],
         "W_qkvh": w_h, "W_qkvl": w_l, "W_fh": wf_h, "W_fl": wf_l,
         "b_qkv": b_qkv,
         "b_v": bv_b, "W_projh": wp_h, "W_projl": wp_l, "W_pb45": wp45,
         "b_pb": bp_b}
        for b in range(B)
    ]
    res = run_bass_kernel_spmd(nc, in_maps, core_ids=list(range(B)))
    return np.stack([np.asarray(res.results[b]["out"]) for b in range(B)])


# revision 61
# speedup vs baseline: 1.0375x; 1.0104x over previous
"""Multi-head attention block on 8 Trainium2 NeuronCores.

Problem: B=8, N=1024, E=768, H=12, D=64 attention (QKV proj -> softmax(QK^T/8)V
-> output proj), fp32 I/O. Data parallel over batch: core b owns batch b.

v4 design (split-fp8 DoubleRow QKV + all-bf16 attention, host preprocessing):
  - Host precomputes transposed split-fp8 x (xh+xl ~= 32*x^T) and split-fp8
    W_qkv (wh+wl ~= 1024*W_qkv), bf16 W_proj / bias rows. All device loads are
    plain HWDGE DMAs (no casts, no SWDGE descriptor generation, no PE
    transposes of x).
  - QKV projection: 9 fp8 DoubleRow matmuls per psum tile ((xh+xl)@(wh+wl)
    with the xl@wl term dropped), 256-deep contraction pairs at 0.5 cyc/row.
    Psum carries 2^15 scale; Q/K evac rescales (tensor_scalar mult+add bias),
    V keeps the scale which cancels against the 2^15 ones-column in Z.
  - S^T[k,q] per head: two 512-wide bf16 matmuls into a [128,1024] psum
    (contraction d=64 at partition base (h%2)*64); exp on Act -> bf16 expS.
  - U restructured: stationary = expS chunk [128k,128q], moving = V [128k,65]
    (64 dims + 2^15 ones column) -> psum U^T[q,65] accumulated over k chunks;
    invZ = reciprocal of column 64 is a per-partition scalar; attn = U*invZ
    is one DVE tensor_scalar op. Halves U's PE rows vs the classic layout
    and kills the PE invZ broadcast.
  - attn rows (token-major) -> attnT (feature-major) via HWDGE XBAR DMA
    transposes (3 [128,128] bf16 blocks per DMA, zero PE cost).
  - Output proj split: attnT blocks 0..2 projected during late attention as
    PE filler; blocks 3..5 in the tail, accumulated into the same SBUF tile.
  - Emission interleaves S psum fills with QK/V/U/proj filler units so the
    Act engine (exp is ~100us of work, the secondary wall) starves as little
    as possible while PE (the primary wall) stays busy.
"""
import numpy as np

B, N, E, H, D = 8, 1024, 768, 12, 64
SCALE = D ** -0.5
NT = N // 128   # token chunks (8)
NE = E // 128   # embed chunks (6)
NQ = N // 512   # moving-dim tiles (2)
NFS = [(0, 512), (512, 256)]  # free-dim split of E for matmuls
PROJ_SPLIT = 3  # attnT blocks 0..2 in projA (during attention), 3..5 in tail


def _build():
    import concourse.bacc as bacc
    import concourse.mybir as mybir
    import concourse.tile as tile

    F32 = mybir.dt.float32
    BF16 = mybir.dt.bfloat16
    F8 = mybir.dt.float8e4
    EXP = mybir.ActivationFunctionType.Exp
    DR = mybir.MatmulPerfMode.DoubleRow
    MUL = mybir.AluOpType.mult
    ADD = mybir.AluOpType.add

    nc = bacc.Bacc("TRN2", target_bir_lowering=False)
    xh_d = nc.declare_dram_parameter("xh", [E, N], F8, isOutput=False)
    xl_d = nc.declare_dram_parameter("xl", [E, N], F8, isOutput=False)
    wqkvh_d = nc.declare_dram_parameter("W_qkvh", [E, 3 * E], F8, isOutput=False)
    wqkvl_d = nc.declare_dram_parameter("W_qkvl", [E, 3 * E], F8, isOutput=False)
    wfh_d = nc.declare_dram_parameter("W_fh", [E, 256], F8, isOutput=False)
    wfl_d = nc.declare_dram_parameter("W_fl", [E, 256], F8, isOutput=False)
    bqkv_d = nc.declare_dram_parameter("b_qkv", [3 * E], F32, isOutput=False)
    bv_d = nc.declare_dram_parameter("b_v", [E], BF16, isOutput=False)
    wprojh_d = nc.declare_dram_parameter("W_projh", [E, E], F8, isOutput=False)
    wprojl_d = nc.declare_dram_parameter("W_projl", [E, E], F8, isOutput=False)
    wpb45_d = nc.declare_dram_parameter("W_pb45", [256, E], BF16, isOutput=False)
    bp_d = nc.declare_dram_parameter("b_pb", [E], BF16, isOutput=False)
    out_d = nc.declare_dram_parameter("out", [N, E], F32, isOutput=True)

    with tile.TileContext(nc) as tc:
        with (
            tc.tile_pool(name="const", bufs=1) as cp,
            tc.tile_pool(name="main", bufs=1) as qp,
            tc.tile_pool(name="psum", bufs=1, space="PSUM") as ps,
        ):
            # ---- constants ----
            ones1 = cp.tile([1, 128], BF16)
            nc.vector.memset(ones1, 1.0)
            ones32k = cp.tile([1, 128], BF16)
            nc.vector.memset(ones32k, 32768.0)
            bqc = cp.tile([128, 12], F32)   # column fc = b_qkv[128fc:128(fc+1)]

            # ---- long-lived tensors ----
            qT = [qp.tile([128, N], BF16, name=f"qT{c}", tag=f"qT{c}")
                  for c in range(6)]
            kT = [qp.tile([128, N], BF16, name=f"kT{c}", tag=f"kT{c}")
                  for c in range(6)]
            vS = [qp.tile([128, 65 * H], BF16, name=f"vS{i}", tag=f"vS{i}")
                  for i in range(NT)]
            attnS = [qp.tile([128, E], BF16, name=f"atS{i}", tag=f"atS{i}")
                     for i in range(NT)]
            attnT = qp.tile([128, NE * N], BF16)  # [128, (c, 1024)] = 32*attn^T
            attnTv = attnT.rearrange("p (c n) -> p c n", n=N)
            attnTh = qp.tile([128, NE * N], F8)
            attnThv = attnTh.rearrange("p (c n) -> p c n", n=N)
            attnTl = qp.tile([128, NE * N], F8)
            attnTlv = attnTl.rearrange("p (c n) -> p c n", n=N)
            # W_proj pair tiles: pair pp = rows 256pp..256pp+256 as [128, 2, E]
            wphB = qp.tile([128, 2 * 2 * E], F8)
            wplB = qp.tile([128, 2 * 2 * E], F8)
            wphv = wphB.rearrange("p (pp t f) -> p pp t f", t=2, f=E)
            wplv = wplB.rearrange("p (pp t f) -> p pp t f", t=2, f=E)
            wpb45 = qp.tile([128, 2 * E], BF16)
            wpb45v = wpb45.rearrange("p (c f) -> p c f", f=E)
            bv_bc = qp.tile([128, E], F32)
            bp_bc = qp.tile([128, E], F32)
            bv_row = qp.tile([1, E], BF16)
            bp_row = qp.tile([1, E], BF16)
            o_acc = [qp.tile([128, E], F32, name=f"oa{i}", tag=f"oa{i}")
                     for i in range(NT)]

            # expS pool: [128, N] bf16 tiles; 3 heads alive (lag 2)
            ep = tc.alloc_tile_pool(name="exp", bufs=1)
            iz = tc.alloc_tile_pool(name="iz", bufs=1)

            # scoped pool: x / W_qkv fp8 tiles, released once QKV is done
            xp = tc.alloc_tile_pool(name="xw", bufs=1)
            xH = xp.tile([128, NE * N], F8)   # [128, (j, 1024 tok)] = 32*x^T
            xL = xp.tile([128, NE * N], F8)
            xHv = xH.rearrange("p (j n) -> p j n", n=N)
            xLv = xL.rearrange("p (j n) -> p j n", n=N)
            # weight pair big tiles: [128, (p, t, f)] with pair p = W rows
            # 256p..256p+256 split as 2 k-subtiles t
            wqkhB = xp.tile([128, 3 * 2 * 1536], F8)
            wqklB = xp.tile([128, 3 * 2 * 1536], F8)
            wqkhv = wqkhB.rearrange("p (pp t f) -> p pp t f", t=2, f=1536)
            wqklv = wqklB.rearrange("p (pp t f) -> p pp t f", t=2, f=1536)
            wqk0h = xp.tile([128, 3 * 2 * 256], F8)
            wqk0l = xp.tile([128, 3 * 2 * 256], F8)
            wqk0hv = wqk0h.rearrange("p (pp t f) -> p pp t f", t=2, f=256)
            wqk0lv = wqk0l.rearrange("p (pp t f) -> p pp t f", t=2, f=256)
            wvhB = xp.tile([128, 3 * 2 * E], F8)
            wvlB = xp.tile([128, 3 * 2 * E], F8)
            wvhv = wvhB.rearrange("p (pp t f) -> p pp t f", t=2, f=E)
            wvlv = wvlB.rearrange("p (pp t f) -> p pp t f", t=2, f=E)

            # ---- DMAs (all HWDGE, no casts): few big transfers, with two
            # small priority slices so the first S unit starts early ----
            def wqk_slice(wview, w_d, c0, cw):
                nc.sync.dma_start(
                    out=wview[:, :, :, c0:c0 + cw],
                    in_=w_d[0:768, c0:c0 + cw].rearrange(
                        "(pp t k) f -> k pp t f", t=2, k=128))

            # 1. host-packed t=0|t=6 weight columns (gate the first S unit)
            for w_t, w_d in ((wqk0h, wfh_d), (wqk0l, wfl_d)):
                nc.sync.dma_start(
                    out=w_t.rearrange("p (pp t f) -> p pp t f", t=2, f=256),
                    in_=w_d[:, :].rearrange(
                        "(pp t k) f -> k pp t f", t=2, k=128))
            # 2. x token-half 0, then half 1
            for half in range(2):
                t0 = half * 512
                nc.sync.dma_start(
                    out=xHv[:, :, t0:t0 + 512],
                    in_=xh_d[:, t0:t0 + 512].rearrange(
                        "(j k) n -> k j n", k=128))
                nc.sync.dma_start(
                    out=xLv[:, :, t0:t0 + 512],
                    in_=xl_d[:, t0:t0 + 512].rearrange(
                        "(j k) n -> k j n", k=128))
            nc.sync.dma_start(
                out=bqc, in_=bqkv_d[0:1536].rearrange("(f p) -> p f", p=128))
            nc.sync.dma_start(
                out=bv_row, in_=bv_d[:].rearrange("(o f) -> o f", o=1))
            nc.sync.dma_start(
                out=bp_row, in_=bp_d[:].rearrange("(o f) -> o f", o=1))
            # 3. remaining wqk columns
            wqk_slice(wqkhv, wqkvh_d, 128, 640)
            wqk_slice(wqklv, wqkvl_d, 128, 640)
            wqk_slice(wqkhv, wqkvh_d, 896, 640)
            wqk_slice(wqklv, wqkvl_d, 896, 640)
            # 4. V weights, bias rows, proj weights
            for w_t, w_d in ((wvhB, wqkvh_d), (wvlB, wqkvl_d)):
                nc.sync.dma_start(
                    out=w_t.rearrange("p (pp t f) -> p pp t f", t=2, f=E),
                    in_=w_d[0:768, 1536:].rearrange(
                        "(pp t k) f -> k pp t f", t=2, k=128))
            for w_t, w_d in ((wphB, wprojh_d), (wplB, wprojl_d)):
                nc.sync.dma_start(
                    out=w_t.rearrange("p (pp t f) -> p pp t f", t=2, f=E),
                    in_=w_d[0:512, :].rearrange(
                        "(pp t k) f -> k pp t f", t=2, k=128))
            nc.sync.dma_start(
                out=wpb45.rearrange("p (c f) -> p c f", f=E),
                in_=wpb45_d[:].rearrange("(c k) f -> k c f", k=128))

            def emit_prelude():
                for nf, (f0, fw) in enumerate(NFS):
                    pbv = ps.tile([128, 512], F32, name=f"pbv{nf}", tag="mm",
                                  bufs=2)
                    nc.tensor.matmul(pbv[:, :fw], ones32k,
                                     bv_row[:, f0:f0 + fw],
                                     start=True, stop=True)
                    nc.vector.tensor_copy(bv_bc[:, f0:f0 + fw], pbv[:, :fw])
                    pbp = ps.tile([128, 512], F32, name=f"pbp{nf}", tag="mm",
                                  bufs=2)
                    nc.tensor.matmul(pbp[:, :fw], ones32k,
                                     bp_row[:, f0:f0 + fw],
                                     start=True, stop=True)
                    nc.vector.tensor_copy(bp_bc[:, f0:f0 + fw], pbp[:, :fw])
                for i in range(NT):
                    nc.vector.memset(
                        vS[i].rearrange("p (h c) -> p h c", c=65)[:, :, 64:65],
                        1024.0)

            # ================= emission units =================
            def emit_qk_unit(t, q, evac_act=False, first=False):
                """One (feature-tile, 512-token-half) of Q or K projection.
                Split-fp8 DoubleRow: (xh+xl)@(wh+wl), xl@wl dropped."""
                dst = qT[t] if t < 6 else kT[t - 6]
                pq = ps.tile([128, 512], F32, name=f"pq{t}_{q}", tag="mm",
                             bufs=2)
                if first:
                    wcol0 = 0 if t == 0 else 128
                    terms = [(wqk0hv, xHv), (wqk0hv, xLv), (wqk0lv, xHv)]
                else:
                    wcol0 = t * 128
                    terms = [(wqkhv, xHv), (wqkhv, xLv), (wqklv, xHv)]
                nmm = 3 * len(terms)
                mi = 0
                for p in range(3):
                    for w_v, x_v in terms:
                        nc.tensor.matmul(
                            pq,
                            w_v[:, p, :, wcol0:wcol0 + 128],
                            x_v[:, 2 * p:2 * p + 2, q * 512:(q + 1) * 512],
                            start=(mi == 0), stop=(mi == nmm - 1),
                            perf_mode=DR)
                        mi += 1
                if evac_act:
                    nc.scalar.activation(
                        dst[:, q * 512:(q + 1) * 512], pq,
                        mybir.ActivationFunctionType.Identity,
                        bias=bqc[:, t:t + 1], scale=float(2.0 ** -15))
                else:
                    nc.vector.tensor_scalar(
                        out=dst[:, q * 512:(q + 1) * 512], in0=pq,
                        scalar1=float(2.0 ** -15), scalar2=bqc[:, t:t + 1],
                        op0=MUL, op1=ADD)

            def emit_v_unit(i, nf):
                """One (token-chunk, free-half) of the V projection."""
                f0, fw = NFS[nf]
                pv = ps.tile([128, 512], F32, name=f"pv{i}_{nf}", tag="mm",
                             bufs=2)
                terms = [(xHv, wvhv), (xHv, wvlv), (xLv, wvhv)]
                nmm = 3 * len(terms)
                mi = 0
                for p in range(3):
                    for x_v, w_v in terms:
                        nc.tensor.matmul(
                            pv[:, :fw],
                            x_v[:, 2 * p:2 * p + 2, i * 128:(i + 1) * 128],
                            w_v[:, p, :, f0:f0 + fw],
                            start=(mi == 0), stop=(mi == nmm - 1),
                            perf_mode=DR)
                        mi += 1
                nh, h0 = fw // D, f0 // D
                nc.vector.tensor_add(
                    vS[i].rearrange("p (h c) -> p h c", c=65)
                        [:, h0:h0 + nh, 0:64],
                    pv[:, :fw].rearrange("p (h d) -> p h d", d=D),
                    bv_bc[:, f0:f0 + fw].rearrange("p (h d) -> p h d", d=D))

            expS_of = {}

            def emit_s_unit(h, kc):
                """S^T[k-chunk, all q] for one head + exp -> bf16 expS."""
                c, r0 = h // 2, (h % 2) * 64
                if kc == 0:
                    expS_of[h] = [
                        ep.tile([128, N], BF16, name=f"eS{h}_{k2}",
                                tag="expS", bufs=24)
                        for k2 in range(NT)]
                pss = ps.tile([128, N], F32, name=f"ps{h}_{kc}", tag="s2",
                              bufs=2)
                for q in range(NQ):
                    nc.tensor.matmul(
                        pss[:, q * 512:(q + 1) * 512],
                        kT[c][r0:r0 + 64, kc * 128:(kc + 1) * 128],
                        qT[c][r0:r0 + 64, q * 512:(q + 1) * 512],
                        start=True, stop=True)
                    if h == 0 and kc == 0:
                        nc.scalar.activation(
                            expS_of[h][kc][:, q * 512:(q + 1) * 512],
                            pss[:, q * 512:(q + 1) * 512], EXP,
                            scale=float(SCALE))
                if not (h == 0 and kc == 0):
                    nc.scalar.activation(expS_of[h][kc], pss, EXP,
                                         scale=float(SCALE))

            def emit_u_unit(h, qc, evac_act=False):
                """U^T[q-chunk, 65] for one head; normalize into attnS."""
                expS = expS_of[h]
                pu = ps.tile([128, 512], F32, name=f"pu{h}_{qc}", tag="u",
                             bufs=2)
                for kc in range(NT):
                    nc.tensor.matmul(
                        pu[:, 0:65],
                        expS[kc][:, qc * 128:(qc + 1) * 128],
                        vS[kc][:, h * 65:h * 65 + 65],
                        start=(kc == 0), stop=(kc == NT - 1))
                invz = iz.tile([128, 1], F32, name=f"iv{h}_{qc}", tag="iz",
                               bufs=3)
                nc.vector.reciprocal(invz, pu[:, 64:65])
                if evac_act:
                    nc.scalar.activation(
                        attnS[qc][:, h * D:(h + 1) * D], pu[:, 0:64],
                        mybir.ActivationFunctionType.Copy, scale=invz)
                else:
                    nc.vector.tensor_scalar_mul(
                        attnS[qc][:, h * D:(h + 1) * D], pu[:, 0:64], invz)

            def emit_transp(qc, pp):
                """DMA-transpose one block pair into attnT (32*attn^T bf16),
                then split to fp8 hi/lo on the Pool engine."""
                c0 = 2 * pp
                t_bf = attnTv[:, c0:c0 + 2, qc * 128:(qc + 1) * 128]
                nc.sync.dma_start_transpose(
                    t_bf, attnS[qc][:, c0 * 128:(c0 + 2) * 128])
                if pp < 2:
                    t_hi = attnThv[:, c0:c0 + 2, qc * 128:(qc + 1) * 128]
                    nc.gpsimd.tensor_copy(t_hi, t_bf)
                    nc.gpsimd.tensor_sub(
                        attnTlv[:, c0:c0 + 2, qc * 128:(qc + 1) * 128],
                        t_bf, t_hi)

            def emit_proj_unit(i, nf, pp):
                """Half-row of output proj for block pair pp (split-fp8 DR).
                pp=0 adds the (2^15-scaled) bias; pp=1 accumulates; pp=2
                accumulates and rescales to the final f32 output."""
                f0, fw = NFS[nf]
                po = ps.tile([128, 512], F32, name=f"po{i}_{nf}_{pp}",
                             tag="mm", bufs=2)
                if pp < 2:
                    terms = [(attnThv, wphv), (attnThv, wplv), (attnTlv, wphv)]
                    for mi, (a_v, w_v) in enumerate(terms):
                        nc.tensor.matmul(
                            po[:, :fw],
                            a_v[:, 2 * pp:2 * pp + 2, i * 128:(i + 1) * 128],
                            w_v[:, pp, :, f0:f0 + fw],
                            start=(mi == 0), stop=(mi == len(terms) - 1),
                            perf_mode=DR)
                else:
                    for ci, cb in enumerate((4, 5)):
                        nc.tensor.matmul(
                            po[:, :fw],
                            attnTv[:, cb, i * 128:(i + 1) * 128],
                            wpb45v[:, cb - 4, f0:f0 + fw],
                            start=(ci == 0), stop=(ci == 1))
                if pp == 0:
                    nc.vector.tensor_add(
                        o_acc[i][:, f0:f0 + fw], po[:, :fw],
                        bp_bc[:, f0:f0 + fw])
                elif pp == 1:
                    nc.vector.tensor_add(
                        o_acc[i][:, f0:f0 + fw], po[:, :fw],
                        o_acc[i][:, f0:f0 + fw])
                    # rescale the pairs-0/1(+bias) partial here, off the tail
                    nc.gpsimd.tensor_scalar_mul(
                        o_acc[i][:, f0:f0 + fw], o_acc[i][:, f0:f0 + fw],
                        float(2.0 ** -15))
                else:
                    nc.vector.tensor_add(
                        o_acc[i][:, f0:f0 + fw], po[:, :fw],
                        o_acc[i][:, f0:f0 + fw])

            # ================= schedule =================
            for q in range(NQ):
                for t in (0, 6):
                    emit_qk_unit(t, q, first=True)

            for h in range(H):
                c = h // 2
                fillers = []
                if h == 0:
                    fillers.append(("prelude",))
                if h < 10:  # QK chunk c+1: 2 units during each of h=2c, 2c+1
                    t0 = (c + 1, 6 + c + 1)
                    if h % 2 == 0:
                        fillers.append(("qk", t0[0], 0))
                        fillers.append(("qk", t0[1], 0))
                    else:
                        fillers.append(("qk", t0[0], 1))
                        fillers.append(("qk", t0[1], 1))
                # V projection: nf0 (heads 0-7 columns, needed by U(0) at
                # h2) early; nf1 (heads 8-11, first needed by U(8) at h10)
                # in the slack heads 4-5.
                if h in (1, 2):
                    lo, hi = (0, 4) if h == 1 else (4, 8)
                    for i in range(lo, hi):
                        fillers.append(("v", i, 0))
                if h in (4, 5, 6):
                    lo, hi = ((0, 3), (3, 6), (6, 8))[h - 4]
                    for i in range(lo, hi):
                        fillers.append(("v", i, 1))
                if h >= 2:  # U for head h-2 (after V units at h=2)
                    for qc in range(NT):
                        fillers.append(("u", h - 2, qc))
                if h == 10:
                    fillers.append(("xfree",))
                if h in (6, 7, 8, 9):  # proj pair 0 (ready after U(3))
                    lo, hi = ((0, 2), (2, 4), (4, 6), (6, 8))[h - 6]
                    for i in range(lo, hi):
                        fillers.append(("pa", i, 0, 0))
                        fillers.append(("pa", i, 1, 0))
                if h in (10, 11):  # proj pair 1 (blocks 2,3; after U(7))
                    lo, hi = (0, 4) if h == 10 else (4, 8)
                    for i in range(lo, hi):
                        fillers.append(("pa", i, 0, 1))
                        fillers.append(("pa", i, 1, 1))
                if h == 11:  # U(10) late in head 11 (exp(10) done by then)
                    for qc in range(NT):
                        fillers.append(("u", 10, qc))

                fillers.sort(
                    key=lambda f: (f[0] == "u", f[0] != "prelude"))

                def drain(k):
                    for _ in range(k):
                        if not fillers:
                            return
                        f = fillers.pop(0)
                        if f[0] == "v":
                            emit_v_unit(f[1], f[2])
                        elif f[0] == "qk":
                            emit_qk_unit(f[1], f[2])
                        elif f[0] == "u":
                            emit_u_unit(f[1], f[2])
                            if f[1] == 3:
                                emit_transp(f[2], 0)
                            elif f[1] == 7:
                                emit_transp(f[2], 1)
                        elif f[0] == "pa":
                            emit_proj_unit(f[1], f[2], f[3])
                        elif f[0] == "xfree":
                            xp.release()
                        elif f[0] == "prelude":
                            emit_prelude()

                for kc in range(NT):
                    emit_s_unit(h, kc)
                    drain((len(fillers) + NT - kc - 1) // (NT - kc))
                drain(len(fillers))

            # ---- tail: U(11) + attnT blocks 4,5 + proj pair 2, pipelined
            def emit_out(i):
                po = ps.tile([128, 1024], F32, name=f"pot{i}", tag="s2",
                             bufs=2)
                for nf, (f0, fw) in enumerate(NFS):
                    for ci, cb in enumerate((4, 5)):
                        nc.tensor.matmul(
                            po[:, f0:f0 + fw],
                            attnTv[:, cb, i * 128:(i + 1) * 128],
                            wpb45v[:, cb - 4, f0:f0 + fw],
                            start=(ci == 0), stop=(ci == 1))
                nc.vector.tensor_add(
                    o_acc[i], po[:, 0:E], o_acc[i])
                nc.sync.dma_start(
                    out=out_d[i * 128:(i + 1) * 128, :], in_=o_acc[i])

            for qc in range(NT):
                emit_u_unit(11, qc)
                emit_transp(qc, 2)
            for qc in range(NT):
                emit_out(qc)
            iz.release()
            ep.release()
    nc.compile()
    return nc


_NC_CACHE = None


def kernel(x, W_qkv, b_qkv, W_proj, b_proj):
    from concourse.bass_utils import run_bass_kernel_spmd
    import ml_dtypes

    F8NP = ml_dtypes.float8_e4m3
    BF16NP = ml_dtypes.bfloat16

    global _NC_CACHE
    if _NC_CACHE is None:
        _NC_CACHE = _build()
    nc = _NC_CACHE

    x = np.asarray(x, dtype=np.float32)
    W_qkv = np.asarray(W_qkv, dtype=np.float32)
    b_qkv = np.ascontiguousarray(np.asarray(b_qkv, dtype=np.float32))
    W_proj = np.asarray(W_proj, dtype=np.float32)
    b_proj = np.asarray(b_proj, dtype=np.float32)

    # host-side preprocessing: transposed split-fp8 x, split-fp8 W_qkv,
    # bf16 W_proj and bias rows
    def split8(a):
        hi = a.astype(F8NP)
        lo = (a - hi.astype(np.float32)).astype(F8NP)
        return np.ascontiguousarray(hi), np.ascontiguousarray(lo)

    w_h, w_l = split8(W_qkv * np.float32(1024.0))
    wf_h = np.ascontiguousarray(
        np.concatenate([w_h[:, 0:128], w_h[:, 768:896]], axis=1))
    wf_l = np.ascontiguousarray(
        np.concatenate([w_l[:, 0:128], w_l[:, 768:896]], axis=1))
    xt32 = np.ascontiguousarray(np.swapaxes(x, 1, 2)) * np.float32(32.0)
    xsplit = [split8(xt32[b]) for b in range(B)]
    wp_h, wp_l = split8(W_proj * np.float32(1024.0))
    wp45 = np.ascontiguousarray(
        (W_proj[512:, :] * np.float32(1.0 / 32.0)).astype(BF16NP))
    bv_b = np.ascontiguousarray(b_qkv[2 * E:].astype(BF16NP))
    bp_b = np.ascontiguousarray(b_proj.astype(BF16NP))

    in_maps = [
        {"xh": xsplit[b][0], "xl": xsplit[b][1],
         "W_qkvh": w_h, "W_qkvl": w_l, "W_fh": wf_h, "W_fl": wf_l,
         "b_qkv": b_qkv,
         "b_v": bv_b, "W_projh": wp_h, "W_projl": wp_l, "W_pb45": wp45,
         "b_pb": bp_b}
        for b in range(B)
    ]
    res = run_bass_kernel_spmd(nc, in_maps, core_ids=list(range(B)))
    return np.stack([np.asarray(res.results[b]["out"]) for b in range(B)])
